# revision 1
# baseline (speedup 1.0000x reference)
"""AlphaRotatedGIoULoss on 8 TRN2 NeuronCores.

Data-parallel: 500000 box pairs sharded 62500/core, laid out as
(125 partitions x 500 boxes). Per-box rotated-GIoU via a branchless
line-integral intersection (slab clipping in each box's axis-aligned
frame + a frame-change correction term), so no sorting/gather is needed.
Output: per-core partial loss sums (125,1); host sums and divides.
"""
import sys
import numpy as np

for _p in ("/opt/trn_rl_repo", "/root/.axon_site/_ro/trn_rl_repo"):
    if _p not in sys.path:
        sys.path.insert(0, _p)

N_CORES = 8
N_TOTAL = 500000
N_CORE = N_TOTAL // N_CORES   # 62500
P = 125                       # partitions used
FB = 500                      # boxes per partition row (125*500 = 62500)
NT = 2                        # column tiles
FT = FB // NT                 # boxes per column tile
PI_2 = 1.5707963267948966

_CACHE = {}


def _build():
    import concourse.bass as bass  # noqa: F401
    import concourse.bacc as bacc
    import concourse.tile as tile
    from concourse import mybir

    f32 = mybir.dt.float32
    AF = mybir.ActivationFunctionType
    OP = mybir.AluOpType
    AX_ = mybir.AxisListType

    import os
    debug = bool(os.environ.get("K_DEBUG"))
    nc = bacc.Bacc(None, target_bir_lowering=False)
    pred_d = nc.declare_dram_parameter("pred", [N_CORE, 5], f32, isOutput=False)
    tgt_d = nc.declare_dram_parameter("target", [N_CORE, 5], f32, isOutput=False)
    out_d = nc.declare_dram_parameter("out", [P, 1], f32, isOutput=True)
    dbg_d = None
    if debug:
        dbg_d = nc.declare_dram_parameter("dbg", [4, P, FB], f32, isOutput=True)

    V = nc.vector
    S = nc.scalar
    G = nc.gpsimd

    def vtt(out, a, b, op):
        V.tensor_tensor(out, a, b, op)

    def gtt(out, a, b, op):
        # GpSimd elementwise proved both slower (Q7 per-instruction overhead
        # at these tile widths) and unreliable here -> everything on VectorE
        V.tensor_tensor(out, a, b, op)

    from contextlib import ExitStack

    with tile.TileContext(nc) as tc:
        with (
            tc.tile_pool(name="pre", bufs=1) as pre,
            tc.tile_pool(name="small", bufs=1) as sm,
            ExitStack() as stack,
        ):
            io = stack.enter_context(tc.tile_pool(name="io", bufs=1))
            comb = io.tile([P, 2 * FB * 5], f32, tag="comb")
            pio2 = sm.tile([P, 1], f32, tag="pio2")
            V.memset(pio2[:], PI_2)
            # 1-elem warm-up: loads the Sin ACT table while the DMA runs
            warm = sm.tile([P, 1], f32, tag="warm")
            S.activation(warm[:], pio2[:], AF.Sin)
            cv = comb[:].rearrange("p (h f c) -> p h f c", h=2, c=5)
            # halves of comb: h=0 pred, h=1 target
            # (an 8-way partition-row DMA split was tried and measured SLOWER:
            # 32-row chunks engage only a quarter of the SBUF ports each)
            nc.sync.dma_start(out=cv[:, 0], in_=pred_d.rearrange("(p f) c -> p f c", p=P))
            nc.sync.dma_start(out=cv[:, 1], in_=tgt_d.rearrange("(p f) c -> p f c", p=P))

            def feat(h, i):       # (P, FB) plain feature plane view
                return cv[:, h, :, i]

            def featS(i):         # (P, 2, FB) stacked [pred|target]
                return cv[:, :, :, i]

            # stacked planes: physical (P, 2*FB); half 0 = frame-B terms
            # (A's geometry clipped by target box B), half 1 = frame-A terms.
            class SP:
                def __init__(self, name, w=FB):
                    self.w = w
                    self.t = pre.tile([P, 2 * w], f32, tag=name)

                def full(self):
                    return self.t[:]

                def h(self, i):
                    return self.t[:, i * self.w:(i + 1) * self.w]

                def sl(self, c0, n):   # (P,2,n) column slice of both halves
                    return self.t[:].rearrange("p (h f) -> p h f", h=2)[:, :, c0:c0 + n]

                def hsl(self, i, c0, n):
                    return self.t[:, i * self.w + c0: i * self.w + c0 + n]

            ddxS, ddyS, dlt = SP("ddxS"), SP("ddyS"), SP("dlt")
            sdS, cdS, cS, sS = SP("sdS"), SP("cdS"), SP("cS"), SP("sS")
            t1p, t2p = SP("t1p"), SP("t2p")
            dX, dY = SP("dX"), SP("dY")
            whS, hhS = SP("whS"), SP("hhS")
            wc, ws, hs, hc = SP("wc"), SP("ws"), SP("hs"), SP("hc")
            g0x, g0y, n1, n2 = SP("g0x"), SP("g0y"), SP("n1"), SP("n2")
            Wc, Hc = SP("Wc"), SP("Hc")
            rwc, rws, rhs, rhc = SP("rwc"), SP("rws"), SP("rhs"), SP("rhc")

            # ---- pre-pass (full width) ----
            gtt(ddxS.h(0), feat(0, 0), feat(1, 0), OP.subtract)   # x1-x2
            gtt(ddxS.h(1), feat(1, 0), feat(0, 0), OP.subtract)   # x2-x1
            gtt(ddyS.h(0), feat(0, 1), feat(1, 1), OP.subtract)
            gtt(ddyS.h(1), feat(1, 1), feat(0, 1), OP.subtract)
            vtt(dlt.h(0), feat(0, 4), feat(1, 4), OP.subtract)    # a1-a2
            vtt(dlt.h(1), feat(1, 4), feat(0, 4), OP.subtract)
            # all Sin activations batched (one table set); cS/sS first so
            # VectorE's delta chain (which consumes them) can start earlier
            S.activation(cS.h(0), feat(1, 4), AF.Sin, bias=pio2[:])      # c2
            S.activation(cS.h(1), feat(0, 4), AF.Sin, bias=pio2[:])      # c1
            S.activation(sS.h(0), feat(1, 4), AF.Sin)                    # s2
            S.activation(sS.h(1), feat(0, 4), AF.Sin)                    # s1
            S.activation(sdS.full(), dlt.full(), AF.Sin)                 # [sd|-sd]
            # cos(dlt) = sin(dlt + pi/2); dlt+pi/2 can exceed pi where the
            # Sin table degrades -> wrap into [-pi, pi] first
            V.add_range_wrap(cdS.full(), dlt.full(), PI_2, 3.141592653589793,
                             6.283185307179586)
            S.activation(cdS.full(), cdS.full(), AF.Sin)                 # [cd|cd]
            # delta = R^T * (center difference), stacked (GpSimd chain)
            gtt(t1p.full(), cS.full(), ddxS.full(), OP.mult)
            gtt(t2p.full(), sS.full(), ddyS.full(), OP.mult)
            gtt(dX.full(), t1p.full(), t2p.full(), OP.add)
            gtt(t1p.full(), cS.full(), ddyS.full(), OP.mult)
            gtt(t2p.full(), sS.full(), ddxS.full(), OP.mult)
            gtt(dY.full(), t1p.full(), t2p.full(), OP.subtract)
            # half dims of the moving box: [w1|w2]/2, [h1|h2]/2
            S.activation(whS.full(), featS(2), AF.Copy, scale=0.5)
            S.activation(hhS.full(), featS(3), AF.Copy, scale=0.5)
            vtt(wc.full(), whS.full(), cdS.full(), OP.mult)
            vtt(ws.full(), whS.full(), sdS.full(), OP.mult)
            vtt(hs.full(), hhS.full(), sdS.full(), OP.mult)
            vtt(hc.full(), hhS.full(), cdS.full(), OP.mult)
            gtt(g0x.full(), wc.full(), hs.full(), OP.subtract)
            gtt(g0y.full(), ws.full(), hc.full(), OP.add)
            gtt(n1.full(), wc.full(), hs.full(), OP.add)          # -g1x
            gtt(n2.full(), hc.full(), ws.full(), OP.subtract)     # g1y
            # clip half-extents of the fixed box: [w2|w1]/2, [h2|h1]/2 (+neg)
            S.activation(Wc.h(0), feat(1, 2), AF.Copy, scale=0.5)
            S.activation(Wc.h(1), feat(0, 2), AF.Copy, scale=0.5)
            S.activation(Hc.h(0), feat(1, 3), AF.Copy, scale=0.5)
            S.activation(Hc.h(1), feat(0, 3), AF.Copy, scale=0.5)
            nWc, nHc = SP("nWc"), SP("nHc")
            S.activation(nWc.h(0), feat(1, 2), AF.Copy, scale=-0.5)
            S.activation(nWc.h(1), feat(0, 2), AF.Copy, scale=-0.5)
            S.activation(nHc.h(0), feat(1, 3), AF.Copy, scale=-0.5)
            S.activation(nHc.h(1), feat(0, 3), AF.Copy, scale=-0.5)
            # reciprocals of edge direction components; the +1e-20 only
            # rescues an exact-zero denominator (parallel edges) from NaN
            for rp, src in ((rwc, wc), (rws, ws), (rhs, hs), (rhc, hc)):
                S.activation(rp.full(), src.full(), AF.Copy, scale=2.0, bias=1e-20)
                V.reciprocal_approx_fast(out=rp.full(), in_=rp.full())
            # union0 = w1*h1 + w2*h2  (plain width FB)
            m1 = io.tile([P, FB], f32, tag="m1")
            m2 = io.tile([P, FB], f32, tag="m2")
            union0 = sm.tile([P, FB], f32, tag="union0")
            gtt(m1[:], feat(0, 2), feat(0, 3), OP.mult)
            gtt(m2[:], feat(1, 2), feat(1, 3), OP.mult)
            gtt(union0[:], m1[:], m2[:], OP.add)

            # input tile + prepass scratch no longer needed: free the io pool
            # so the heavy per-column-tile pool can use its SBUF space
            stack.close()
            hv = stack.enter_context(tc.tile_pool(name="heavy", bufs=1))

            lsums = []
            SW = 2 * FT  # stacked width per edge slice

            for t in range(NT):
                c0 = t * FT

                def E(tile4):     # (P, 4, 2, FT) edge/half view of 4*SW tile
                    return tile4[:].rearrange("p (e h f) -> p e h f", e=4, h=2)

                AXt = hv.tile([P, 4 * SW], f32, tag="AXt")
                AYt = hv.tile([P, 4 * SW], f32, tag="AYt")
                INX = hv.tile([P, 4 * SW], f32, tag="INX")
                INY = hv.tile([P, 4 * SW], f32, tag="INY")
                Ut = hv.tile([P, 4 * SW], f32, tag="Ut")
                Vt = hv.tile([P, 4 * SW], f32, tag="Vt")
                NPt = hv.tile([P, 4 * SW], f32, tag="NPt")
                TLX = hv.tile([P, 4 * SW], f32, tag="TLX")

                dXc, dYc = dX.sl(c0, FT), dY.sl(c0, FT)
                g0xc, g0yc = g0x.sl(c0, FT), g0y.sl(c0, FT)
                n1c, n2c = n1.sl(c0, FT), n2.sl(c0, FT)
                Wcc, Hcc = Wc.sl(c0, FT), Hc.sl(c0, FT)

                # corners of the moving box in the fixed box's frame (GpSimd)
                gtt(E(AXt)[:, 0], dXc, g0xc, OP.add)
                gtt(E(AXt)[:, 1], dXc, n1c, OP.subtract)
                gtt(E(AXt)[:, 2], dXc, g0xc, OP.subtract)
                gtt(E(AXt)[:, 3], dXc, n1c, OP.add)
                gtt(E(AYt)[:, 0], dYc, g0yc, OP.add)
                gtt(E(AYt)[:, 1], dYc, n2c, OP.add)
                gtt(E(AYt)[:, 2], dYc, g0yc, OP.subtract)
                gtt(E(AYt)[:, 3], dYc, n2c, OP.subtract)

                # ---- enclosing rect (bbox in each frame, min of the two) ----
                exm = sm.tile([P, SW], f32, tag="exm")
                exn = sm.tile([P, SW], f32, tag="exn")
                exs = sm.tile([P, SW], f32, tag="exs")
                eys = sm.tile([P, SW], f32, tag="eys")
                ex3 = exm[:].rearrange("p (h f) -> p h f", h=2)
                en3 = exn[:].rearrange("p (h f) -> p h f", h=2)
                es3 = exs[:].rearrange("p (h f) -> p h f", h=2)
                ey3 = eys[:].rearrange("p (h f) -> p h f", h=2)
                # (min/max TT are not legal on GpSimd; comparisons stay on V,
                # adds/mults go to GpSimd)
                for crn, ext, clamp, dst3 in ((AXt, exs, Wcc, es3), (AYt, eys, Hcc, ey3)):
                    c4 = E(crn)
                    vtt(ex3, c4[:, 0], c4[:, 1], OP.max)
                    vtt(en3, c4[:, 2], c4[:, 3], OP.max)
                    vtt(ex3, ex3, en3, OP.max)                    # mx
                    vtt(dst3, ex3, clamp, OP.max)                 # max(mx, W)
                    vtt(ex3, c4[:, 0], c4[:, 1], OP.min)
                    vtt(en3, c4[:, 2], c4[:, 3], OP.min)
                    vtt(ex3, ex3, en3, OP.min)                    # mn
                    S.activation(ex3, ex3, AF.Copy, scale=-1.0)   # -mn
                    vtt(ex3, ex3, clamp, OP.max)                  # max(-mn, W)
                    gtt(dst3, dst3, ex3, OP.add)                  # extent
                gtt(exs[:], exs[:], eys[:], OP.mult)              # areaC stacked
                area_c = sm.tile([P, FT], f32, tag="area_c")
                vtt(area_c[:], es3[:, 0], es3[:, 1], OP.min)

                # ---- signed reciprocal planes (edges 0,1 only: edges 2,3
                # come from the point symmetry a2 = 2*delta - a0, d2 = -d0,
                # whose slab roots are m + roots(edge0), m = 2*delta*inv) ----
                rwcc, rwsc = rwc.sl(c0, FT), rws.sl(c0, FT)
                rhsc, rhcc = rhs.sl(c0, FT), rhc.sl(c0, FT)
                for dst, srcs in (
                    (INX, ((rwcc, -1.0), (rhsc, 1.0))),
                    (INY, ((rwsc, -1.0), (rhcc, -1.0))),
                ):
                    d4 = E(dst)
                    for e, (src, sc) in enumerate(srcs):
                        S.activation(d4[:, e], src, AF.Copy, scale=sc)
                t2d = sm.tile([P, SW], f32, tag="t2d")
                t2d3 = t2d[:].rearrange("p (h f) -> p h f", h=2)
                HW2 = 2 * SW

                def H01(t4):
                    return t4[:, 0:HW2]

                def H23(t4):
                    return t4[:, HW2:2 * HW2]

                # ---- slab clip, x axis, edges 0,1 ----
                nWcc, nHcc = nWc.sl(c0, FT), nHc.sl(c0, FT)
                for e in range(2):
                    vtt(E(Ut)[:, e], nWcc, E(AXt)[:, e], OP.subtract)  # -W - ax
                    vtt(E(Vt)[:, e], Wcc, E(AXt)[:, e], OP.subtract)   # W - ax
                vtt(H01(Ut), H01(Ut), H01(INX), OP.mult)               # ta01
                vtt(H01(Vt), H01(Vt), H01(INX), OP.mult)               # tb01
                vtt(H01(TLX), H01(Ut), H01(Vt), OP.min)                # tlo01
                vtt(H01(Ut), H01(Ut), H01(Vt), OP.max)                 # thi01
                S.activation(t2d3, dXc, AF.Copy, scale=2.0)            # 2*dx
                vtt(E(Vt)[:, 0], t2d3, E(INX)[:, 0], OP.mult)          # m0
                vtt(E(Vt)[:, 1], t2d3, E(INX)[:, 1], OP.mult)          # m1
                vtt(H23(TLX), H01(Vt), H01(TLX), OP.add)               # tlo23
                vtt(H23(Ut), H01(Vt), H01(Ut), OP.add)                 # thi23
                # ---- slab clip, y axis, edges 0,1 ----
                for e in range(2):
                    vtt(E(Vt)[:, e], nHcc, E(AYt)[:, e], OP.subtract)  # -H - ay
                    vtt(E(NPt)[:, e], Hcc, E(AYt)[:, e], OP.subtract)  # H - ay
                vtt(H01(Vt), H01(Vt), H01(INY), OP.mult)               # ta01_y
                vtt(H01(NPt), H01(NPt), H01(INY), OP.mult)             # tb01_y
                vtt(H01(INX), H01(Vt), H01(NPt), OP.min)               # tlo01_y
                vtt(H01(Vt), H01(Vt), H01(NPt), OP.max)                # thi01_y
                S.activation(t2d3, dYc, AF.Copy, scale=2.0)            # 2*dy
                vtt(E(NPt)[:, 0], t2d3, E(INY)[:, 0], OP.mult)         # m0_y
                vtt(E(NPt)[:, 1], t2d3, E(INY)[:, 1], OP.mult)         # m1_y
                vtt(H23(INX), H01(NPt), H01(INX), OP.add)              # tlo23_y
                vtt(H23(Vt), H01(NPt), H01(Vt), OP.add)                # thi23_y
                # ---- interval intersect, dt ----
                # t0 = max(tlo_x, tlo_y, 0); t1 = min(thi_x, thi_y, 1)
                V.scalar_tensor_tensor(TLX[:], TLX[:], 0.0, INX[:], OP.max, OP.max)
                V.scalar_tensor_tensor(Ut[:], Ut[:], 1.0, Vt[:], OP.min, OP.min)
                vtt(TLX[:], Ut[:], TLX[:], OP.subtract)                # t1-t0
                S.activation(TLX[:], TLX[:], AF.Relu)                  # dt
                # ---- direction planes, cross(a,d), pieces ----
                wcc, wsc = wc.sl(c0, FT), ws.sl(c0, FT)
                hsc, hcc = hs.sl(c0, FT), hc.sl(c0, FT)
                for dst, srcs in (
                    (INX, ((wcc, -2.0), (hsc, 2.0), (wcc, 2.0), (hsc, -2.0))),   # dx
                    (INY, ((wsc, -2.0), (hcc, -2.0), (wsc, 2.0), (hcc, 2.0))),   # dy
                ):
                    d4 = E(dst)
                    for e, (src, sc) in enumerate(srcs):
                        S.activation(d4[:, e], src, AF.Copy, scale=sc)
                vtt(Vt[:], AXt[:], INY[:], OP.mult)                    # ax*dy
                vtt(Ut[:], AYt[:], INX[:], OP.mult)                    # ay*dx
                vtt(Vt[:], Vt[:], Ut[:], OP.subtract)                  # cad
                vtt(Ut[:], TLX[:], Vt[:], OP.mult)                     # pieces

                # ---- piece sum (stacked), SA (frame-B half) ----
                psS = sm.tile([P, SW], f32, tag="psS")
                ps3 = psS[:].rearrange("p (h f) -> p h f", h=2)
                u4 = E(Ut)
                gtt(ps3, u4[:, 0], u4[:, 1], OP.add)
                gtt(es3, u4[:, 2], u4[:, 3], OP.add)                   # reuse exs
                gtt(ps3, ps3, es3, OP.add)
                dt4 = E(TLX)
                sax = sm.tile([P, FT], f32, tag="sax")
                say = sm.tile([P, FT], f32, tag="say")
                sau = sm.tile([P, FT], f32, tag="sau")
                sav = sm.tile([P, FT], f32, tag="sav")
                st1 = sm.tile([P, FT], f32, tag="st1")
                gtt(sau[:], dt4[:, 2, 0], dt4[:, 0, 0], OP.subtract)
                gtt(sav[:], dt4[:, 3, 0], dt4[:, 1, 0], OP.subtract)
                dx4, dy4 = E(INX), E(INY)
                gtt(sax[:], dx4[:, 2, 0], sau[:], OP.mult)
                gtt(st1[:], dx4[:, 3, 0], sav[:], OP.mult)
                gtt(sax[:], sax[:], st1[:], OP.add)
                gtt(say[:], dy4[:, 2, 0], sau[:], OP.mult)
                gtt(st1[:], dy4[:, 3, 0], sav[:], OP.mult)
                gtt(say[:], say[:], st1[:], OP.add)
                # corr = ddy*(c2*sax - s2*say)... rotated by R2:
                # RSx = c2*sax - s2*say ; RSy = s2*sax + c2*say
                c2v = cS.hsl(0, c0, FT)
                s2v = sS.hsl(0, c0, FT)
                rsx = sm.tile([P, FT], f32, tag="rsx")
                rsy = sm.tile([P, FT], f32, tag="rsy")
                gtt(rsx[:], c2v, sax[:], OP.mult)
                gtt(st1[:], s2v, say[:], OP.mult)
                gtt(rsx[:], rsx[:], st1[:], OP.subtract)
                gtt(rsy[:], s2v, sax[:], OP.mult)
                gtt(st1[:], c2v, say[:], OP.mult)
                gtt(rsy[:], rsy[:], st1[:], OP.add)
                inter = sm.tile([P, FT], f32, tag="inter")
                gtt(inter[:], ddyS.hsl(0, c0, FT), rsx[:], OP.mult)
                gtt(st1[:], ddxS.hsl(0, c0, FT), rsy[:], OP.mult)
                gtt(inter[:], inter[:], st1[:], OP.subtract)           # corr
                gtt(inter[:], inter[:], ps3[:, 0], OP.add)
                gtt(inter[:], inter[:], ps3[:, 1], OP.add)
                S.activation(inter[:], inter[:], AF.Relu, scale=0.5)   # inter area

                # ---- final loss ----
                union = sm.tile([P, FT], f32, tag="union")
                gtt(union[:], union0[:, c0:c0 + FT], inter[:], OP.subtract)
                iou = sm.tile([P, FT], f32, tag="iou")
                V.reciprocal_approx_fast(out=st1[:], in_=union[:])
                gtt(iou[:], inter[:], st1[:], OP.mult)
                V.tensor_scalar(iou[:], iou[:], 1e-6, None, OP.max)
                V.reciprocal_approx_fast(out=st1[:], in_=area_c[:])
                gtt(st1[:], union[:], st1[:], OP.mult)
                rr = sm.tile([P, FT], f32, tag="rr")
                S.activation(rr[:], st1[:], AF.Copy, scale=-1.0, bias=1.0)  # 1-u/ac
                gtt(st1[:], iou[:], iou[:], OP.mult)
                gtt(st1[:], st1[:], iou[:], OP.mult)                   # iou^3
                gtt(iou[:], rr[:], rr[:], OP.mult)
                gtt(iou[:], iou[:], rr[:], OP.mult)                    # r^3
                gtt(st1[:], st1[:], iou[:], OP.subtract)               # giou
                S.activation(st1[:], st1[:], AF.Copy, scale=-1.0, bias=1.0)
                ls = sm.tile([P, 1], f32, tag=f"ls{t}")
                V.tensor_reduce(ls[:], st1[:], AX_.X, OP.add)
                if debug:
                    nc.sync.dma_start(out=dbg_d[0, :, c0:c0 + FT], in_=st1[:])
                    nc.sync.dma_start(out=dbg_d[1, :, c0:c0 + FT], in_=inter[:])
                    nc.sync.dma_start(out=dbg_d[2, :, c0:c0 + FT], in_=union[:])
                    nc.sync.dma_start(out=dbg_d[3, :, c0:c0 + FT], in_=area_c[:])
                lsums.append(ls)

            acc = sm.tile([P, 1], f32, tag="acc")
            gtt(acc[:], lsums[0][:], lsums[1][:], OP.add)
            nc.sync.dma_start(out=out_d[:], in_=acc[:])

    nc.finalize()
    return nc


def _get_nc():
    if "nc" not in _CACHE:
        _CACHE["nc"] = _build()
    return _CACHE["nc"]


def kernel(pred, target):
    from concourse.bass_utils import run_bass_kernel_spmd

    pred = np.ascontiguousarray(np.asarray(pred, dtype=np.float32))
    target = np.ascontiguousarray(np.asarray(target, dtype=np.float32))
    nc = _get_nc()
    in_maps = []
    for i in range(N_CORES):
        sl = slice(i * N_CORE, (i + 1) * N_CORE)
        in_maps.append({"pred": pred[sl], "target": target[sl]})
    res = run_bass_kernel_spmd(nc, in_maps, core_ids=list(range(N_CORES)))
    total = np.float64(0.0)
    for i in range(N_CORES):
        total += np.asarray(res.results[i]["out"], dtype=np.float64).sum()
    return np.float32(total / N_TOTAL)



# revision 3
# speedup vs baseline: 1.4412x; 1.4412x over previous
"""AlphaRotatedGIoULoss on 8 TRN2 NeuronCores.

Data-parallel: 500000 box pairs sharded 62500/core, laid out as
(125 partitions x 500 boxes). Per-box rotated-GIoU via a branchless
line-integral intersection (slab clipping in each box's axis-aligned
frame + a frame-change correction term), so no sorting/gather is needed.

v2: heavy elementwise chain in fp16 (DVE 2x_1p mode = 2x throughput),
with geometry pre-scaled by 1/16 (folded into existing scale factors)
so all products stay in fp16 range; reciprocal slab planes are clamped
to +-3e4 before the fp16 convert so 0*inf NaNs cannot occur. The final
per-box loss section stays fp32 (reciprocal_approx_fast is fp32-only).
Output: per-core partial giou sums (125,1); host computes 1 - sum/N.
"""
import sys
import numpy as np

for _p in ("/opt/trn_rl_repo", "/root/.axon_site/_ro/trn_rl_repo"):
    if _p not in sys.path:
        sys.path.insert(0, _p)

N_CORES = 8
N_TOTAL = 500000
N_CORE = N_TOTAL // N_CORES   # 62500
P = 125                       # partitions used
FB = 500                      # boxes per partition row (125*500 = 62500)
SW = 2 * FB                   # stacked width (both halves)
PI_2 = 1.5707963267948966
SC = 1.0 / 16.0               # global geometry scale (power of 2, exact)
CL = 30000.0                  # fp16-safe clamp for reciprocal planes

_CACHE = {}


def _build():
    import concourse.bass as bass  # noqa: F401
    import concourse.bacc as bacc
    import concourse.tile as tile
    from concourse import mybir

    f32 = mybir.dt.float32
    f16 = mybir.dt.float16
    AF = mybir.ActivationFunctionType
    OP = mybir.AluOpType
    AX_ = mybir.AxisListType

    import os
    debug = bool(os.environ.get("K_DEBUG"))
    nc = bacc.Bacc(None, target_bir_lowering=False)
    pred_d = nc.declare_dram_parameter("pred", [N_CORE, 5], f32, isOutput=False)
    tgt_d = nc.declare_dram_parameter("target", [N_CORE, 5], f32, isOutput=False)
    out_d = nc.declare_dram_parameter("out", [P, 1], f32, isOutput=True)
    dbg_d = None
    if debug:
        dbg_d = nc.declare_dram_parameter("dbg", [4, P, FB], f32, isOutput=True)

    V = nc.vector
    S = nc.scalar

    def vtt(out, a, b, op):
        V.tensor_tensor(out, a, b, op)

    from contextlib import ExitStack

    with tile.TileContext(nc) as tc:
        with (
            tc.tile_pool(name="pre", bufs=1) as pre,
            tc.tile_pool(name="small", bufs=1) as sm,
            ExitStack() as stack,
        ):
            io = stack.enter_context(tc.tile_pool(name="io", bufs=1))
            comb = io.tile([P, 2 * FB * 5], f32, tag="comb")
            pio2 = sm.tile([P, 1], f32, tag="pio2")
            V.memset(pio2[:], PI_2)
            # 1-elem warm-up: loads the Sin ACT table while the DMA runs
            warm = sm.tile([P, 1], f32, tag="warm")
            S.activation(warm[:], pio2[:], AF.Sin)
            cv = comb[:].rearrange("p (h f c) -> p h f c", h=2, c=5)
            # halves of comb: h=0 pred, h=1 target
            nc.sync.dma_start(out=cv[:, 0], in_=pred_d.rearrange("(p f) c -> p f c", p=P))
            nc.sync.dma_start(out=cv[:, 1], in_=tgt_d.rearrange("(p f) c -> p f c", p=P))

            def feat(h, i):       # (P, FB) plain feature plane view
                return cv[:, h, :, i]

            def featS(i):         # (P, 2, FB) stacked [pred|target]
                return cv[:, :, :, i]

            # stacked planes: physical (P, 2*FB); half 0 = frame-B terms
            # (A's geometry clipped by target box B), half 1 = frame-A terms.
            class SP:
                def __init__(self, name, dt=f16, w=FB):
                    self.w = w
                    self.t = pre.tile([P, 2 * w], dt, tag=name)

                def full(self):
                    return self.t[:]

                def h(self, i):
                    return self.t[:, i * self.w:(i + 1) * self.w]

                def v3(self):     # (P, 2, w) stacked view
                    return self.t[:].rearrange("p (h f) -> p h f", h=2)

            ddxS, ddyS = SP("ddxS"), SP("ddyS")
            dx16, dy16 = SP("dx16"), SP("dy16")
            dlt, dltw = SP("dlt", f32), SP("dltw", f32)
            sdS, cdS, cS, sS = SP("sdS"), SP("cdS"), SP("cS"), SP("sS")
            csS, ssS = SP("csS"), SP("ssS")
            t1p, t2p = SP("t1p"), SP("t2p")
            dX, dY = SP("dX"), SP("dY")
            dXm, dYm = SP("dXm"), SP("dYm")
            whS, hhS = SP("whS"), SP("hhS")
            wc, ws, hs, hc = SP("wc"), SP("ws"), SP("hs"), SP("hc")
            g0x, g0y, n1, n2 = SP("g0x"), SP("g0y"), SP("n1"), SP("n2")
            Wc, Hc, nWc, nHc = SP("Wc"), SP("Hc"), SP("nWc"), SP("nHc")
            rp32a, rp32b = SP("rp32a", f32), SP("rp32b", f32)
            # persistent pre-signed clamped reciprocal planes, (P, 2e, 2h, FB)
            rIX = pre.tile([P, 2 * SW], f16, tag="rIX")
            rIY = pre.tile([P, 2 * SW], f16, tag="rIY")
            rIXe = rIX[:].rearrange("p (e h f) -> p e h f", e=2, h=2)
            rIYe = rIY[:].rearrange("p (e h f) -> p e h f", e=2, h=2)

            # ---- pre-pass (full width) ----
            vtt(ddxS.h(0), feat(0, 0), feat(1, 0), OP.subtract)   # x1-x2 (f16)
            V.tensor_scalar(ddxS.h(1), ddxS.h(0), -1.0, None, OP.mult)
            vtt(ddyS.h(0), feat(0, 1), feat(1, 1), OP.subtract)
            V.tensor_scalar(ddyS.h(1), ddyS.h(0), -1.0, None, OP.mult)
            vtt(dlt.h(0), feat(0, 4), feat(1, 4), OP.subtract)    # a1-a2 (f32)
            V.tensor_scalar(dlt.h(1), dlt.h(0), -1.0, None, OP.mult)
            # all Sin activations batched (one table set)
            S.activation(cS.h(0), feat(1, 4), AF.Sin, bias=pio2[:])      # c2
            S.activation(cS.h(1), feat(0, 4), AF.Sin, bias=pio2[:])      # c1
            S.activation(sS.h(0), feat(1, 4), AF.Sin)                    # s2
            S.activation(sS.h(1), feat(0, 4), AF.Sin)                    # s1
            S.activation(sdS.full(), dlt.full(), AF.Sin)                 # [sd|-sd]
            # cos(dlt) = sin(dlt + pi/2); wrap into [-pi, pi] first
            V.add_range_wrap(dltw.full(), dlt.full(), PI_2, 3.141592653589793,
                             6.283185307179586)
            S.activation(cdS.full(), dltw.full(), AF.Sin)                # [cd|cd]
            # 1/16-scaled trig copies: carry the geometry scale into dX/dY
            V.tensor_scalar(csS.full(), cS.full(), SC, None, OP.mult)
            V.tensor_scalar(ssS.full(), sS.full(), SC, None, OP.mult)
            V.tensor_scalar(dx16.full(), ddxS.full(), SC, None, OP.mult)
            V.tensor_scalar(dy16.full(), ddyS.full(), SC, None, OP.mult)
            # delta = R^T * (center difference)/16, stacked
            vtt(t1p.full(), csS.full(), ddxS.full(), OP.mult)
            vtt(t2p.full(), ssS.full(), ddyS.full(), OP.mult)
            vtt(dX.full(), t1p.full(), t2p.full(), OP.add)
            vtt(t1p.full(), csS.full(), ddyS.full(), OP.mult)
            vtt(t2p.full(), ssS.full(), ddxS.full(), OP.mult)
            vtt(dY.full(), t1p.full(), t2p.full(), OP.subtract)
            V.tensor_scalar(dXm.full(), dX.full(), 2.0, None, OP.mult)   # 2*dx
            V.tensor_scalar(dYm.full(), dY.full(), 2.0, None, OP.mult)
            # half dims of the moving box, /16: [w1|w2]/32, [h1|h2]/32
            S.activation(whS.full(), featS(2), AF.Copy, scale=0.5 * SC)
            S.activation(hhS.full(), featS(3), AF.Copy, scale=0.5 * SC)
            vtt(wc.full(), whS.full(), cdS.full(), OP.mult)
            vtt(ws.full(), whS.full(), sdS.full(), OP.mult)
            vtt(hs.full(), hhS.full(), sdS.full(), OP.mult)
            vtt(hc.full(), hhS.full(), cdS.full(), OP.mult)
            vtt(g0x.full(), wc.full(), hs.full(), OP.subtract)
            vtt(g0y.full(), ws.full(), hc.full(), OP.add)
            vtt(n1.full(), wc.full(), hs.full(), OP.add)          # -g1x
            vtt(n2.full(), hc.full(), ws.full(), OP.subtract)     # g1y
            # clip half-extents of the fixed box, /16 (+neg)
            S.activation(Wc.h(0), feat(1, 2), AF.Copy, scale=0.5 * SC)
            S.activation(Wc.h(1), feat(0, 2), AF.Copy, scale=0.5 * SC)
            S.activation(Hc.h(0), feat(1, 3), AF.Copy, scale=0.5 * SC)
            S.activation(Hc.h(1), feat(0, 3), AF.Copy, scale=0.5 * SC)
            S.activation(nWc.h(0), feat(1, 2), AF.Copy, scale=-0.5 * SC)
            S.activation(nWc.h(1), feat(0, 2), AF.Copy, scale=-0.5 * SC)
            S.activation(nHc.h(0), feat(1, 3), AF.Copy, scale=-0.5 * SC)
            S.activation(nHc.h(1), feat(0, 3), AF.Copy, scale=-0.5 * SC)
            # pre-signed reciprocal slab planes: rIX e0 = -1/(2wc),
            # e1 = +1/(2hs); rIY e0 = -1/(2ws), e1 = -1/(2hc).
            # sign folded into the ACT scale; clamp to +-CL then fp16.
            for (dst, src, sgn, rp) in (
                (rIXe[:, 0], wc, -1.0, rp32a),
                (rIXe[:, 1], hs, 1.0, rp32b),
                (rIYe[:, 0], ws, -1.0, rp32a),
                (rIYe[:, 1], hc, -1.0, rp32b),
            ):
                S.activation(rp.full(), src.full(), AF.Copy,
                             scale=2.0 * sgn, bias=1e-20 * sgn)
                V.reciprocal_approx_fast(out=rp.full(), in_=rp.full())
                V.tensor_scalar(dst, rp.v3(), CL, -CL, OP.min, OP.max)
            # union0 = (w1h1 + w2h2)/1024 via whS*hhS; the *4 to reach the
            # /256 scale of inter is folded into the final union STT
            u01 = sm.tile([P, SW], f16, tag="u01")
            union0 = sm.tile([P, FB], f32, tag="union0")
            vtt(u01[:], whS.full(), hhS.full(), OP.mult)
            u013 = u01[:].rearrange("p (h f) -> p h f", h=2)
            vtt(union0[:], u013[:, 0], u013[:, 1], OP.add)

            # input tile no longer needed: free the io pool
            stack.close()
            hv = stack.enter_context(tc.tile_pool(name="heavy", bufs=1))

            def E(tile4):     # (P, 4, 2, FB) edge/half view of 4*SW tile
                return tile4[:].rearrange("p (e h f) -> p e h f", e=4, h=2)

            AXt = hv.tile([P, 4 * SW], f16, tag="AXt")
            AYt = hv.tile([P, 4 * SW], f16, tag="AYt")
            DRX = hv.tile([P, 4 * SW], f16, tag="DRX")
            DRY = hv.tile([P, 4 * SW], f16, tag="DRY")
            Ut = hv.tile([P, 4 * SW], f16, tag="Ut")
            Vt = hv.tile([P, 4 * SW], f16, tag="Vt")
            NPt = hv.tile([P, 4 * SW], f16, tag="NPt")
            TLX = hv.tile([P, 4 * SW], f16, tag="TLX")

            # corners of the moving box in the fixed box's frame
            vtt(E(AXt)[:, 0], dX.v3(), g0x.v3(), OP.add)
            vtt(E(AXt)[:, 1], dX.v3(), n1.v3(), OP.subtract)
            vtt(E(AXt)[:, 2], dX.v3(), g0x.v3(), OP.subtract)
            vtt(E(AXt)[:, 3], dX.v3(), n1.v3(), OP.add)
            vtt(E(AYt)[:, 0], dY.v3(), g0y.v3(), OP.add)
            vtt(E(AYt)[:, 1], dY.v3(), n2.v3(), OP.add)
            vtt(E(AYt)[:, 2], dY.v3(), g0y.v3(), OP.subtract)
            vtt(E(AYt)[:, 3], dY.v3(), n2.v3(), OP.subtract)

            # ---- enclosing rect (bbox in each frame, min of the two) ----
            exm = sm.tile([P, SW], f16, tag="exm")
            exn = sm.tile([P, SW], f16, tag="exn")
            exs = sm.tile([P, SW], f16, tag="exs")
            eys = sm.tile([P, SW], f16, tag="eys")
            ex3 = exm[:].rearrange("p (h f) -> p h f", h=2)
            en3 = exn[:].rearrange("p (h f) -> p h f", h=2)
            es3 = exs[:].rearrange("p (h f) -> p h f", h=2)
            ey3 = eys[:].rearrange("p (h f) -> p h f", h=2)
            for crn, clamp, dst3 in ((AXt, Wc, es3), (AYt, Hc, ey3)):
                c4 = E(crn)
                vtt(ex3, c4[:, 0], c4[:, 1], OP.max)
                vtt(en3, c4[:, 2], c4[:, 3], OP.max)
                vtt(ex3, ex3, en3, OP.max)                    # mx
                vtt(dst3, ex3, clamp.v3(), OP.max)            # max(mx, W)
                vtt(ex3, c4[:, 0], c4[:, 1], OP.min)
                vtt(en3, c4[:, 2], c4[:, 3], OP.min)
                vtt(ex3, ex3, en3, OP.min)                    # mn
                V.tensor_scalar(ex3, ex3, -1.0, None, OP.mult)
                vtt(ex3, ex3, clamp.v3(), OP.max)             # max(-mn, W)
                vtt(dst3, dst3, ex3, OP.add)                  # extent
            vtt(exs[:], exs[:], eys[:], OP.mult)              # areaC stacked
            area_c = sm.tile([P, FB], f32, tag="area_c")
            vtt(area_c[:], es3[:, 0], es3[:, 1], OP.min)

            HW2 = 2 * SW

            def H01(t4):
                return t4[:, 0:HW2]

            def H23(t4):
                return t4[:, HW2:2 * HW2]

            # ---- slab clip, x axis, edges 0,1 (2,3 via point symmetry:
            # roots(edge2) = m + roots(edge0), m = -2*dX*inx = dXm*rIX) ----
            for e in range(2):
                vtt(E(Ut)[:, e], nWc.v3(), E(AXt)[:, e], OP.subtract)  # -W - ax
                vtt(E(Vt)[:, e], Wc.v3(), E(AXt)[:, e], OP.subtract)   # W - ax
            vtt(H01(Ut), H01(Ut), rIX[:], OP.mult)                 # ta01
            vtt(H01(Vt), H01(Vt), rIX[:], OP.mult)                 # tb01
            vtt(H01(TLX), H01(Ut), H01(Vt), OP.min)                # tlo01
            vtt(H01(Ut), H01(Ut), H01(Vt), OP.max)                 # thi01
            vtt(E(Vt)[:, 0], dXm.v3(), rIXe[:, 0], OP.mult)        # m0
            vtt(E(Vt)[:, 1], dXm.v3(), rIXe[:, 1], OP.mult)        # m1
            vtt(H23(TLX), H01(Vt), H01(TLX), OP.add)               # tlo23
            vtt(H23(Ut), H01(Vt), H01(Ut), OP.add)                 # thi23
            # ---- slab clip, y axis, edges 0,1 ----
            for e in range(2):
                vtt(E(Vt)[:, e], nHc.v3(), E(AYt)[:, e], OP.subtract)  # -H - ay
                vtt(E(NPt)[:, e], Hc.v3(), E(AYt)[:, e], OP.subtract)  # H - ay
            vtt(H01(Vt), H01(Vt), rIY[:], OP.mult)                 # ta01_y
            vtt(H01(NPt), H01(NPt), rIY[:], OP.mult)               # tb01_y
            vtt(H01(DRX), H01(Vt), H01(NPt), OP.min)               # tlo01_y
            vtt(H01(Vt), H01(Vt), H01(NPt), OP.max)                # thi01_y
            vtt(E(NPt)[:, 0], dYm.v3(), rIYe[:, 0], OP.mult)       # m0_y
            vtt(E(NPt)[:, 1], dYm.v3(), rIYe[:, 1], OP.mult)       # m1_y
            vtt(H23(DRX), H01(NPt), H01(DRX), OP.add)              # tlo23_y
            vtt(H23(Vt), H01(NPt), H01(Vt), OP.add)                # thi23_y
            # ---- interval intersect, dt ----
            # t0 = max(tlo_x, tlo_y, 0); t1 = min(thi_x, thi_y, 1)
            V.scalar_tensor_tensor(TLX[:], TLX[:], 0.0, DRX[:], OP.max, OP.max)
            V.scalar_tensor_tensor(Ut[:], Ut[:], 1.0, Vt[:], OP.min, OP.min)
            vtt(TLX[:], Ut[:], TLX[:], OP.subtract)                # t1-t0
            S.activation(TLX[:], TLX[:], AF.Relu)                  # dt
            # ---- direction planes (on Scalar), cross(a,d), pieces ----
            for dst, srcs in (
                (DRX, ((wc, -2.0), (hs, 2.0), (wc, 2.0), (hs, -2.0))),   # dx
                (DRY, ((ws, -2.0), (hc, -2.0), (ws, 2.0), (hc, 2.0))),   # dy
            ):
                d4 = E(dst)
                for e, (src, sc) in enumerate(srcs):
                    S.activation(d4[:, e], src.v3(), AF.Copy, scale=sc)
            vtt(Vt[:], AXt[:], DRY[:], OP.mult)                    # ax*dy
            vtt(NPt[:], AYt[:], DRX[:], OP.mult)                   # ay*dx
            vtt(Vt[:], Vt[:], NPt[:], OP.subtract)                 # cad
            vtt(Ut[:], TLX[:], Vt[:], OP.mult)                     # pieces

            # ---- piece sum (stacked), SA correction (frame-B half) ----
            psS = sm.tile([P, SW], f16, tag="psS")
            ps3 = psS[:].rearrange("p (h f) -> p h f", h=2)
            u4 = E(Ut)
            vtt(ps3, u4[:, 0], u4[:, 1], OP.add)
            vtt(es3, u4[:, 2], u4[:, 3], OP.add)                   # reuse exs
            vtt(ps3, ps3, es3, OP.add)
            dt4 = E(TLX)
            sax = sm.tile([P, FB], f16, tag="sax")
            say = sm.tile([P, FB], f16, tag="say")
            sau = sm.tile([P, FB], f16, tag="sau")
            sav = sm.tile([P, FB], f16, tag="sav")
            st1 = sm.tile([P, FB], f16, tag="st1")
            vtt(sau[:], dt4[:, 2, 0], dt4[:, 0, 0], OP.subtract)
            vtt(sav[:], dt4[:, 3, 0], dt4[:, 1, 0], OP.subtract)
            dx4, dy4 = E(DRX), E(DRY)
            vtt(sax[:], dx4[:, 2, 0], sau[:], OP.mult)
            vtt(st1[:], dx4[:, 3, 0], sav[:], OP.mult)
            vtt(sax[:], sax[:], st1[:], OP.add)
            vtt(say[:], dy4[:, 2, 0], sau[:], OP.mult)
            vtt(st1[:], dy4[:, 3, 0], sav[:], OP.mult)
            vtt(say[:], say[:], st1[:], OP.add)
            # corr = ddy/16*(c2*sax - s2*say) - ddx/16*(s2*sax + c2*say)
            c2v = cS.h(0)
            s2v = sS.h(0)
            rsx = sm.tile([P, FB], f16, tag="rsx")
            rsy = sm.tile([P, FB], f16, tag="rsy")
            vtt(rsx[:], c2v, sax[:], OP.mult)
            vtt(st1[:], s2v, say[:], OP.mult)
            vtt(rsx[:], rsx[:], st1[:], OP.subtract)
            vtt(rsy[:], s2v, sax[:], OP.mult)
            vtt(st1[:], c2v, say[:], OP.mult)
            vtt(rsy[:], rsy[:], st1[:], OP.add)
            inter16 = sm.tile([P, FB], f16, tag="inter16")
            vtt(inter16[:], dy16.h(0), rsx[:], OP.mult)
            vtt(st1[:], dx16.h(0), rsy[:], OP.mult)
            vtt(inter16[:], inter16[:], st1[:], OP.subtract)       # corr
            vtt(inter16[:], inter16[:], ps3[:, 0], OP.add)
            vtt(inter16[:], inter16[:], ps3[:, 1], OP.add)
            inter = sm.tile([P, FB], f32, tag="inter")
            S.activation(inter[:], inter16[:], AF.Relu, scale=0.5)  # inter area

            # ---- final loss (fp32) ----
            union = sm.tile([P, FB], f32, tag="union")
            fr1 = sm.tile([P, FB], f32, tag="fr1")
            iou = sm.tile([P, FB], f32, tag="iou")
            rr = sm.tile([P, FB], f32, tag="rr")
            # union = 4*union0 - inter  (the *4 restores the /256 scale)
            V.scalar_tensor_tensor(union[:], union0[:], 4.0, inter[:],
                                   OP.mult, OP.subtract)
            V.reciprocal_approx_fast(out=fr1[:], in_=union[:])
            vtt(iou[:], inter[:], fr1[:], OP.mult)
            V.tensor_scalar(iou[:], iou[:], 1e-6, None, OP.max)
            V.reciprocal_approx_fast(out=fr1[:], in_=area_c[:])
            vtt(fr1[:], union[:], fr1[:], OP.mult)
            S.activation(rr[:], fr1[:], AF.Copy, scale=-1.0, bias=1.0)  # 1-u/ac
            vtt(fr1[:], iou[:], iou[:], OP.mult)
            vtt(fr1[:], fr1[:], iou[:], OP.mult)                   # iou^3
            vtt(iou[:], rr[:], rr[:], OP.mult)
            vtt(iou[:], iou[:], rr[:], OP.mult)                    # r^3
            vtt(fr1[:], fr1[:], iou[:], OP.subtract)               # giou
            ls = sm.tile([P, 1], f32, tag="ls")
            V.tensor_reduce(ls[:], fr1[:], AX_.X, OP.add)          # sum(giou)
            if debug:
                nc.sync.dma_start(out=dbg_d[0], in_=fr1[:])
                nc.sync.dma_start(out=dbg_d[1], in_=inter[:])
                nc.sync.dma_start(out=dbg_d[2], in_=union[:])
                nc.sync.dma_start(out=dbg_d[3], in_=area_c[:])
            nc.sync.dma_start(out=out_d[:], in_=ls[:])

    nc.finalize()
    return nc


def _get_nc():
    if "nc" not in _CACHE:
        _CACHE["nc"] = _build()
    return _CACHE["nc"]


def kernel(pred, target):
    from concourse.bass_utils import run_bass_kernel_spmd

    pred = np.ascontiguousarray(np.asarray(pred, dtype=np.float32))
    target = np.ascontiguousarray(np.asarray(target, dtype=np.float32))
    nc = _get_nc()
    in_maps = []
    for i in range(N_CORES):
        sl = slice(i * N_CORE, (i + 1) * N_CORE)
        in_maps.append({"pred": pred[sl], "target": target[sl]})
    res = run_bass_kernel_spmd(nc, in_maps, core_ids=list(range(N_CORES)))
    gsum = np.float64(0.0)
    for i in range(N_CORES):
        gsum += np.asarray(res.results[i]["out"], dtype=np.float64).sum()
    # loss = mean(1 - giou) = 1 - sum(giou)/N
    return np.float32(1.0 - gsum / N_TOTAL)


# revision 5
# speedup vs baseline: 1.4967x; 1.0385x over previous
"""AlphaRotatedGIoULoss on 8 TRN2 NeuronCores.

Data-parallel: 500000 box pairs sharded 62500/core, laid out as
(125 partitions x 500 boxes). Per-box rotated-GIoU via a branchless
line-integral intersection (slab clipping in each box's axis-aligned
frame + a frame-change correction term), so no sorting/gather is needed.

v3: heavy elementwise chain in fp16 (DVE 2x_1p mode = 2x throughput),
geometry pre-scaled by 1/16 (folded into existing scale factors) so all
products stay in fp16 range; reciprocal slab planes clamped to +-3e4 so
0*inf NaNs cannot occur. Inputs repacked host-side: the angle columns
arrive as a small first DMA so the Sin chain starts ~4us in; the xywh
columns follow as dense (n,4) rows. Prepass scale/copy ACTs moved from
Scalar to cheap Vector tensor_scalar ops (fp16 TS runs 4x) to unblock
the startup. Enclosing-box extents use half_extent_x = |wc|+|hs|.
Output: per-core partial sum(iou^3) - sum(rr^3); host: 1 - sum/N.
"""
import sys
import numpy as np

for _p in ("/opt/trn_rl_repo", "/root/.axon_site/_ro/trn_rl_repo"):
    if _p not in sys.path:
        sys.path.insert(0, _p)

N_CORES = 8
N_TOTAL = 500000
N_CORE = N_TOTAL // N_CORES   # 62500
P = 125                       # partitions used
FB = 500                      # boxes per partition row (125*500 = 62500)
SW = 2 * FB                   # stacked width (both halves)
PI_2 = 1.5707963267948966
SC = 1.0 / 16.0               # global geometry scale (power of 2, exact)
CL = 30000.0                  # fp16-safe clamp for reciprocal planes

_CACHE = {}


def _build():
    import concourse.bass as bass  # noqa: F401
    import concourse.bacc as bacc
    import concourse.tile as tile
    from concourse import mybir

    f32 = mybir.dt.float32
    f16 = mybir.dt.float16
    AF = mybir.ActivationFunctionType
    OP = mybir.AluOpType
    import os
    debug = bool(os.environ.get("K_DEBUG"))
    nc = bacc.Bacc(None, target_bir_lowering=False)
    ang_d = nc.declare_dram_parameter("ang", [2, N_CORE], f32, isOutput=False)
    pred_d = nc.declare_dram_parameter("pred4", [N_CORE, 4], f32, isOutput=False)
    tgt_d = nc.declare_dram_parameter("tgt4", [N_CORE, 4], f32, isOutput=False)
    out_d = nc.declare_dram_parameter("out", [P, 1], f32, isOutput=True)
    dbg_d = None
    if debug:
        dbg_d = nc.declare_dram_parameter("dbg", [4, P, FB], f32, isOutput=True)

    V = nc.vector
    S = nc.scalar

    def vtt(out, a, b, op):
        V.tensor_tensor(out, a, b, op)

    def vts(out, in_, s1, s2, op0, op1=None):
        if op1 is None:
            V.tensor_scalar(out, in_, s1, None, op0)
        else:
            V.tensor_scalar(out, in_, s1, s2, op0, op1)

    from contextlib import ExitStack

    with tile.TileContext(nc) as tc:
        with (
            tc.tile_pool(name="pre", bufs=1) as pre,
            tc.tile_pool(name="small", bufs=1) as sm,
            ExitStack() as stack,
        ):
            io = stack.enter_context(tc.tile_pool(name="io", bufs=1))
            angT = io.tile([P, 2 * FB], f32, tag="angT")
            comb = io.tile([P, 2 * FB * 4], f32, tag="comb")
            pio2 = sm.tile([P, 1], f32, tag="pio2")
            V.memset(pio2[:], PI_2)
            # 1-elem warm-up: loads the Sin ACT table while the DMA runs
            warm = sm.tile([P, 1], f32, tag="warm")
            S.activation(warm[:], pio2[:], AF.Sin)
            angV = angT[:].rearrange("p (h f) -> p h f", h=2)
            cv = comb[:].rearrange("p (h f c) -> p h f c", h=2, c=4)
            # angles first (small, unblocks the Sin chain), then xywh
            nc.sync.dma_start(out=angV, in_=ang_d.rearrange("h (p f) -> p h f", p=P))
            nc.sync.dma_start(out=cv[:, 0], in_=pred_d.rearrange("(p f) c -> p f c", p=P))
            nc.sync.dma_start(out=cv[:, 1], in_=tgt_d.rearrange("(p f) c -> p f c", p=P))

            def feat(h, i):       # (P, FB) xywh feature plane view
                return cv[:, h, :, i]

            def featS(i):         # (P, 2, FB) stacked [pred|target]
                return cv[:, :, :, i]

            def angf(h):
                return angV[:, h]

            class SP:
                def __init__(self, name, dt=f16, w=FB):
                    self.w = w
                    self.t = pre.tile([P, 2 * w], dt, tag=name)

                def full(self):
                    return self.t[:]

                def h(self, i):
                    return self.t[:, i * self.w:(i + 1) * self.w]

                def v3(self):     # (P, 2, w) stacked view
                    return self.t[:].rearrange("p (h f) -> p h f", h=2)

            ddxS, ddyS = SP("ddxS"), SP("ddyS")
            dx16, dy16 = SP("dx16"), SP("dy16")
            dlt, dltw = SP("dlt", f32), SP("dltw", f32)
            sdS, cdS, cS, sS = SP("sdS"), SP("cdS"), SP("cS"), SP("sS")
            csS, ssS = SP("csS"), SP("ssS")
            t1p, t2p = SP("t1p"), SP("t2p")
            dX, dY = SP("dX"), SP("dY")
            dXm, dYm = SP("dXm"), SP("dYm")
            whS, hhS = SP("whS"), SP("hhS")
            wc, ws, hs, hc = SP("wc"), SP("ws"), SP("hs"), SP("hc")
            g0x, g0y, n1, n2 = SP("g0x"), SP("g0y"), SP("n1"), SP("n2")
            Wc, Hc, nWc, nHc = SP("Wc"), SP("Hc"), SP("nWc"), SP("nHc")
            exP, eyP = SP("exP"), SP("eyP")
            aw, ah = SP("aw"), SP("ah")
            rp32a, rp32b = SP("rp32a", f32), SP("rp32b", f32)
            # persistent pre-signed clamped reciprocal planes, (P, 2e, 2h, FB)
            rIX = pre.tile([P, 2 * SW], f16, tag="rIX")
            rIY = pre.tile([P, 2 * SW], f16, tag="rIY")
            rIXe = rIX[:].rearrange("p (e h f) -> p e h f", e=2, h=2)
            rIYe = rIY[:].rearrange("p (e h f) -> p e h f", e=2, h=2)

            # ---- pre-pass, angle part (only needs angT) ----
            vtt(dlt.h(0), angf(0), angf(1), OP.subtract)          # a1-a2 (f32)
            vts(dlt.h(1), dlt.h(0), -1.0, None, OP.mult)
            S.activation(cS.h(0), angf(1), AF.Sin, bias=pio2[:])  # c2
            S.activation(cS.h(1), angf(0), AF.Sin, bias=pio2[:])  # c1
            S.activation(sS.h(0), angf(1), AF.Sin)                # s2
            S.activation(sS.h(1), angf(0), AF.Sin)                # s1
            S.activation(sdS.full(), dlt.full(), AF.Sin)          # [sd|-sd]
            # cos(dlt) = sin(dlt + pi/2); wrap into [-pi, pi] first
            V.add_range_wrap(dltw.full(), dlt.full(), PI_2, 3.141592653589793,
                             6.283185307179586)
            S.activation(cdS.full(), dltw.full(), AF.Sin)         # [cd|cd]
            # 1/16-scaled trig copies carry the geometry scale into dX/dY
            vts(csS.full(), cS.full(), SC, None, OP.mult)
            vts(ssS.full(), sS.full(), SC, None, OP.mult)

            # ---- pre-pass, xywh part ----
            vtt(ddxS.h(0), feat(0, 0), feat(1, 0), OP.subtract)   # x1-x2 (f16)
            vts(ddxS.h(1), ddxS.h(0), -1.0, None, OP.mult)
            vtt(ddyS.h(0), feat(0, 1), feat(1, 1), OP.subtract)
            vts(ddyS.h(1), ddyS.h(0), -1.0, None, OP.mult)
            vts(dx16.full(), ddxS.full(), SC, None, OP.mult)
            vts(dy16.full(), ddyS.full(), SC, None, OP.mult)
            # delta = R^T * (center difference)/16, stacked
            vtt(t1p.full(), csS.full(), ddxS.full(), OP.mult)
            vtt(t2p.full(), ssS.full(), ddyS.full(), OP.mult)
            vtt(dX.full(), t1p.full(), t2p.full(), OP.add)
            vtt(t1p.full(), csS.full(), ddyS.full(), OP.mult)
            vtt(t2p.full(), ssS.full(), ddxS.full(), OP.mult)
            vtt(dY.full(), t1p.full(), t2p.full(), OP.subtract)
            vts(dXm.full(), dX.full(), 2.0, None, OP.mult)        # 2*dx
            vts(dYm.full(), dY.full(), 2.0, None, OP.mult)
            # half dims of the moving box, /16: [w1|w2]/32, [h1|h2]/32
            vts(whS.full(), featS(2), 0.5 * SC, None, OP.mult)
            vts(hhS.full(), featS(3), 0.5 * SC, None, OP.mult)
            vtt(wc.full(), whS.full(), cdS.full(), OP.mult)
            vtt(ws.full(), whS.full(), sdS.full(), OP.mult)
            vtt(hs.full(), hhS.full(), sdS.full(), OP.mult)
            vtt(hc.full(), hhS.full(), cdS.full(), OP.mult)
            vtt(g0x.full(), wc.full(), hs.full(), OP.subtract)
            vtt(g0y.full(), ws.full(), hc.full(), OP.add)
            vtt(n1.full(), wc.full(), hs.full(), OP.add)          # -g1x
            vtt(n2.full(), hc.full(), ws.full(), OP.subtract)     # g1y
            # clip half-extents of the fixed box, /16 (+neg)
            vts(Wc.h(0), feat(1, 2), 0.5 * SC, None, OP.mult)
            vts(Wc.h(1), feat(0, 2), 0.5 * SC, None, OP.mult)
            vts(Hc.h(0), feat(1, 3), 0.5 * SC, None, OP.mult)
            vts(Hc.h(1), feat(0, 3), 0.5 * SC, None, OP.mult)
            vts(nWc.full(), Wc.full(), -1.0, None, OP.mult)
            vts(nHc.full(), Hc.full(), -1.0, None, OP.mult)
            # half-extents of the moving box's bbox in the fixed frame:
            # ex = |wc|+|hs|, ey = |ws|+|hc| (corners sit at +-g0x, +-n1
            # around dX and max(|wc-hs|,|wc+hs|) = |wc|+|hs|)
            S.activation(aw.full(), wc.full(), AF.Abs)
            S.activation(ah.full(), hs.full(), AF.Abs)
            vtt(exP.full(), aw.full(), ah.full(), OP.add)
            S.activation(aw.full(), ws.full(), AF.Abs)
            S.activation(ah.full(), hc.full(), AF.Abs)
            vtt(eyP.full(), aw.full(), ah.full(), OP.add)
            # pre-signed reciprocal slab planes: rIX e0 = -1/(2wc),
            # e1 = +1/(2hs); rIY e0 = -1/(2ws), e1 = -1/(2hc).
            # clamp to +-CL then fp16 so 0*inf NaNs cannot occur.
            for (dst, src, sgn, rp) in (
                (rIXe[:, 0], wc, -1.0, rp32a),
                (rIXe[:, 1], hs, 1.0, rp32b),
                (rIYe[:, 0], ws, -1.0, rp32a),
                (rIYe[:, 1], hc, -1.0, rp32b),
            ):
                vts(rp.full(), src.full(), 2.0 * sgn, 1e-20 * sgn, OP.mult, OP.add)
                V.reciprocal_approx_fast(out=rp.full(), in_=rp.full())
                vts(dst, rp.v3(), CL, -CL, OP.min, OP.max)
            # union0 = (w1h1 + w2h2)/1024; the *4 to reach the /256 scale of
            # inter is folded into the final union STT
            u01 = sm.tile([P, SW], f16, tag="u01")
            union0 = sm.tile([P, FB], f32, tag="union0")
            vtt(u01[:], whS.full(), hhS.full(), OP.mult)
            u013 = u01[:].rearrange("p (h f) -> p h f", h=2)
            vtt(union0[:], u013[:, 0], u013[:, 1], OP.add)

            # input tiles no longer needed: free the io pool
            stack.close()
            hv = stack.enter_context(tc.tile_pool(name="heavy", bufs=1))

            def E(tile4):     # (P, 4, 2, FB) edge/half view of 4*SW tile
                return tile4[:].rearrange("p (e h f) -> p e h f", e=4, h=2)

            AXt = hv.tile([P, 4 * SW], f16, tag="AXt")
            AYt = hv.tile([P, 4 * SW], f16, tag="AYt")
            DRX = hv.tile([P, 4 * SW], f16, tag="DRX")
            DRY = hv.tile([P, 4 * SW], f16, tag="DRY")
            Ut = hv.tile([P, 4 * SW], f16, tag="Ut")
            Vt = hv.tile([P, 4 * SW], f16, tag="Vt")
            NPt = hv.tile([P, 4 * SW], f16, tag="NPt")
            TLX = hv.tile([P, 4 * SW], f16, tag="TLX")

            # corners of the moving box in the fixed box's frame
            vtt(E(AXt)[:, 0], dX.v3(), g0x.v3(), OP.add)
            vtt(E(AXt)[:, 1], dX.v3(), n1.v3(), OP.subtract)
            vtt(E(AXt)[:, 2], dX.v3(), g0x.v3(), OP.subtract)
            vtt(E(AXt)[:, 3], dX.v3(), n1.v3(), OP.add)
            vtt(E(AYt)[:, 0], dY.v3(), g0y.v3(), OP.add)
            vtt(E(AYt)[:, 1], dY.v3(), n2.v3(), OP.add)
            vtt(E(AYt)[:, 2], dY.v3(), g0y.v3(), OP.subtract)
            vtt(E(AYt)[:, 3], dY.v3(), n2.v3(), OP.subtract)

            # ---- enclosing rect (bbox in each frame, min of the two) ----
            exm = sm.tile([P, SW], f16, tag="exm")
            exn = sm.tile([P, SW], f16, tag="exn")
            exs = sm.tile([P, SW], f16, tag="exs")
            eys = sm.tile([P, SW], f16, tag="eys")
            ex3 = exm[:].rearrange("p (h f) -> p h f", h=2)
            en3 = exn[:].rearrange("p (h f) -> p h f", h=2)
            es3 = exs[:].rearrange("p (h f) -> p h f", h=2)
            ey3 = eys[:].rearrange("p (h f) -> p h f", h=2)
            for ext, d3, clamp, dst3 in ((exP, dX, Wc, es3), (eyP, dY, Hc, ey3)):
                vtt(ex3, d3.v3(), ext.v3(), OP.add)               # dX + ex
                vtt(en3, ext.v3(), d3.v3(), OP.subtract)          # ex - dX
                vtt(ex3, ex3, clamp.v3(), OP.max)
                vtt(en3, en3, clamp.v3(), OP.max)
                vtt(dst3, ex3, en3, OP.add)                       # extent
            vtt(exs[:], exs[:], eys[:], OP.mult)                  # areaC stacked
            area_c = sm.tile([P, FB], f32, tag="area_c")
            vtt(area_c[:], es3[:, 0], es3[:, 1], OP.min)

            HW2 = 2 * SW

            def H01(t4):
                return t4[:, 0:HW2]

            def H23(t4):
                return t4[:, HW2:2 * HW2]

            # ---- slab clip, x axis, edges 0,1 (2,3 via point symmetry:
            # roots(edge2) = m + roots(edge0), m = dXm*rIX) ----
            for e in range(2):
                vtt(E(Ut)[:, e], nWc.v3(), E(AXt)[:, e], OP.subtract)  # -W - ax
                vtt(E(Vt)[:, e], Wc.v3(), E(AXt)[:, e], OP.subtract)   # W - ax
            vtt(H01(Ut), H01(Ut), rIX[:], OP.mult)                 # ta01
            vtt(H01(Vt), H01(Vt), rIX[:], OP.mult)                 # tb01
            vtt(H01(TLX), H01(Ut), H01(Vt), OP.min)                # tlo01
            vtt(H01(Ut), H01(Ut), H01(Vt), OP.max)                 # thi01
            vtt(E(Vt)[:, 0], dXm.v3(), rIXe[:, 0], OP.mult)        # m0
            vtt(E(Vt)[:, 1], dXm.v3(), rIXe[:, 1], OP.mult)        # m1
            vtt(H23(TLX), H01(Vt), H01(TLX), OP.add)               # tlo23
            vtt(H23(Ut), H01(Vt), H01(Ut), OP.add)                 # thi23
            # ---- slab clip, y axis, edges 0,1 ----
            for e in range(2):
                vtt(E(Vt)[:, e], nHc.v3(), E(AYt)[:, e], OP.subtract)  # -H - ay
                vtt(E(NPt)[:, e], Hc.v3(), E(AYt)[:, e], OP.subtract)  # H - ay
            vtt(H01(Vt), H01(Vt), rIY[:], OP.mult)                 # ta01_y
            vtt(H01(NPt), H01(NPt), rIY[:], OP.mult)               # tb01_y
            vtt(H01(DRX), H01(Vt), H01(NPt), OP.min)               # tlo01_y
            vtt(H01(Vt), H01(Vt), H01(NPt), OP.max)                # thi01_y
            vtt(E(NPt)[:, 0], dYm.v3(), rIYe[:, 0], OP.mult)       # m0_y
            vtt(E(NPt)[:, 1], dYm.v3(), rIYe[:, 1], OP.mult)       # m1_y
            vtt(H23(DRX), H01(NPt), H01(DRX), OP.add)              # tlo23_y
            vtt(H23(Vt), H01(NPt), H01(Vt), OP.add)                # thi23_y
            # ---- interval intersect, dt ----
            # t0 = max(tlo_x, tlo_y, 0); t1 = min(thi_x, thi_y, 1)
            vtt(TLX[:], TLX[:], DRX[:], OP.max)
            vts(TLX[:], TLX[:], 0.0, None, OP.max)
            vtt(Ut[:], Ut[:], Vt[:], OP.min)
            vts(Ut[:], Ut[:], 1.0, None, OP.min)
            vtt(TLX[:], Ut[:], TLX[:], OP.subtract)                # t1-t0
            S.activation(TLX[:], TLX[:], AF.Relu)                  # dt
            # ---- direction planes (on Scalar), cross(a,d), pieces ----
            for dst, srcs in (
                (DRX, ((wc, -2.0), (hs, 2.0), (wc, 2.0), (hs, -2.0))),   # dx
                (DRY, ((ws, -2.0), (hc, -2.0), (ws, 2.0), (hc, 2.0))),   # dy
            ):
                d4 = E(dst)
                for e, (src, sc) in enumerate(srcs):
                    S.activation(d4[:, e], src.v3(), AF.Copy, scale=sc)
            vtt(Vt[:], AXt[:], DRY[:], OP.mult)                    # ax*dy
            vtt(NPt[:], AYt[:], DRX[:], OP.mult)                   # ay*dx
            vtt(Vt[:], Vt[:], NPt[:], OP.subtract)                 # cad
            vtt(Ut[:], TLX[:], Vt[:], OP.mult)                     # pieces

            # ---- piece sum (stacked), SA correction (frame-B half) ----
            psS = sm.tile([P, SW], f16, tag="psS")
            ps3 = psS[:].rearrange("p (h f) -> p h f", h=2)
            u4 = E(Ut)
            vtt(ps3, u4[:, 0], u4[:, 1], OP.add)
            vtt(es3, u4[:, 2], u4[:, 3], OP.add)                   # reuse exs
            vtt(ps3, ps3, es3, OP.add)
            dt4 = E(TLX)
            sax = sm.tile([P, FB], f16, tag="sax")
            say = sm.tile([P, FB], f16, tag="say")
            sau = sm.tile([P, FB], f16, tag="sau")
            sav = sm.tile([P, FB], f16, tag="sav")
            st1 = sm.tile([P, FB], f16, tag="st1")
            vtt(sau[:], dt4[:, 2, 0], dt4[:, 0, 0], OP.subtract)
            vtt(sav[:], dt4[:, 3, 0], dt4[:, 1, 0], OP.subtract)
            dx4, dy4 = E(DRX), E(DRY)
            vtt(sax[:], dx4[:, 2, 0], sau[:], OP.mult)
            vtt(st1[:], dx4[:, 3, 0], sav[:], OP.mult)
            vtt(sax[:], sax[:], st1[:], OP.add)
            vtt(say[:], dy4[:, 2, 0], sau[:], OP.mult)
            vtt(st1[:], dy4[:, 3, 0], sav[:], OP.mult)
            vtt(say[:], say[:], st1[:], OP.add)
            # corr = ddy/16*(c2*sax - s2*say) - ddx/16*(s2*sax + c2*say)
            c2v = cS.h(0)
            s2v = sS.h(0)
            rsx = sm.tile([P, FB], f16, tag="rsx")
            rsy = sm.tile([P, FB], f16, tag="rsy")
            vtt(rsx[:], c2v, sax[:], OP.mult)
            vtt(st1[:], s2v, say[:], OP.mult)
            vtt(rsx[:], rsx[:], st1[:], OP.subtract)
            vtt(rsy[:], s2v, sax[:], OP.mult)
            vtt(st1[:], c2v, say[:], OP.mult)
            vtt(rsy[:], rsy[:], st1[:], OP.add)
            inter16 = sm.tile([P, FB], f16, tag="inter16")
            vtt(inter16[:], dy16.h(0), rsx[:], OP.mult)
            vtt(st1[:], dx16.h(0), rsy[:], OP.mult)
            vtt(inter16[:], inter16[:], st1[:], OP.subtract)       # corr
            vtt(inter16[:], inter16[:], ps3[:, 0], OP.add)
            vtt(inter16[:], inter16[:], ps3[:, 1], OP.add)
            inter = sm.tile([P, FB], f32, tag="inter")
            S.activation(inter[:], inter16[:], AF.Relu, scale=0.5)  # inter area

            # ---- final loss (fp32) ----
            union = sm.tile([P, FB], f32, tag="union")
            fr1 = sm.tile([P, FB], f32, tag="fr1")
            iou = sm.tile([P, FB], f32, tag="iou")
            rr = sm.tile([P, FB], f32, tag="rr")
            lsa = sm.tile([P, 1], f32, tag="lsa")
            lsb = sm.tile([P, 1], f32, tag="lsb")
            # union = 4*union0 - inter  (the *4 restores the /256 scale)
            V.scalar_tensor_tensor(union[:], union0[:], 4.0, inter[:],
                                   OP.mult, OP.subtract)
            V.reciprocal_approx_fast(out=fr1[:], in_=union[:])
            vtt(iou[:], inter[:], fr1[:], OP.mult)
            vts(iou[:], iou[:], 1e-6, None, OP.max)
            V.reciprocal_approx_fast(out=fr1[:], in_=area_c[:])
            vtt(fr1[:], union[:], fr1[:], OP.mult)
            vts(rr[:], fr1[:], -1.0, 1.0, OP.mult, OP.add)         # 1 - u/ac
            AXL = mybir.AxisListType
            vtt(fr1[:], iou[:], iou[:], OP.mult)                   # iou^2
            vtt(fr1[:], fr1[:], iou[:], OP.mult)                   # iou^3
            vtt(iou[:], rr[:], rr[:], OP.mult)                     # rr^2
            vtt(iou[:], iou[:], rr[:], OP.mult)                    # rr^3
            vtt(fr1[:], fr1[:], iou[:], OP.subtract)               # giou
            V.tensor_reduce(lsa[:], fr1[:], AXL.X, OP.add)         # sum giou
            if debug:
                nc.sync.dma_start(out=dbg_d[0], in_=inter[:])
                nc.sync.dma_start(out=dbg_d[1], in_=union[:])
                nc.sync.dma_start(out=dbg_d[2], in_=iou[:])
                nc.sync.dma_start(out=dbg_d[3], in_=area_c[:])
            nc.sync.dma_start(out=out_d[:], in_=lsa[:])

    nc.finalize()
    return nc


def _get_nc():
    if "nc" not in _CACHE:
        _CACHE["nc"] = _build()
    return _CACHE["nc"]


def _repack(pred, target):
    """Per-core input repack: angles as a contiguous (2, n) block (small,
    DMA'd first), xywh as dense (n, 4) rows."""
    in_maps = []
    for i in range(N_CORES):
        sl = slice(i * N_CORE, (i + 1) * N_CORE)
        p, t = pred[sl], target[sl]
        ang = np.ascontiguousarray(np.stack([p[:, 4], t[:, 4]], axis=0))
        in_maps.append({
            "ang": ang,
            "pred4": np.ascontiguousarray(p[:, :4]),
            "tgt4": np.ascontiguousarray(t[:, :4]),
        })
    return in_maps


def kernel(pred, target):
    from concourse.bass_utils import run_bass_kernel_spmd

    pred = np.ascontiguousarray(np.asarray(pred, dtype=np.float32))
    target = np.ascontiguousarray(np.asarray(target, dtype=np.float32))
    nc = _get_nc()
    in_maps = _repack(pred, target)
    res = run_bass_kernel_spmd(nc, in_maps, core_ids=list(range(N_CORES)))
    gsum = np.float64(0.0)
    for i in range(N_CORES):
        gsum += np.asarray(res.results[i]["out"], dtype=np.float64).sum()
    # loss = mean(1 - giou) = 1 - sum(giou)/N
    return np.float32(1.0 - gsum / N_TOTAL)


# revision 9
# speedup vs baseline: 1.6745x; 1.1188x over previous
"""AlphaRotatedGIoULoss on 8 TRN2 NeuronCores.

Data-parallel: 500000 box pairs sharded 62500/core, laid out as
(125 partitions x 500 boxes). Per-box rotated-GIoU via a branchless
line-integral intersection (slab clipping in each box's axis-aligned
frame + a frame-change correction term), so no sorting/gather is needed.

v4: heavy elementwise chain in fp16 (DVE 2x_1p mode = 2x throughput),
geometry pre-scaled by 1/16 (folded into existing scale factors) so all
products stay in fp16 range; reciprocal slab planes clamped to +-3e4 so
0*inf NaNs cannot occur. Host repack: angle and w/h columns are cast to
fp16 (error ~0.06px, far under tolerance) and shipped as planar rows so
every SBUF slice is packed; xy stays fp32 for exact center differences.
DMA order ang -> wh -> xy unblocks the Sin chain ~2us in. Point-symmetry
(corner e2,e3 = 2*dX - e0,e1) is exploited with stride-0 broadcast APs
to merge op pairs into single wide DVE passes. Enclosing-box extents
use half_extent_x = |wc|+|hs|. Output: per-core sum(giou); host 1-s/N.
"""
import sys
import numpy as np

for _p in ("/opt/trn_rl_repo", "/root/.axon_site/_ro/trn_rl_repo"):
    if _p not in sys.path:
        sys.path.insert(0, _p)

N_CORES = 8
N_TOTAL = 500000
N_CORE = N_TOTAL // N_CORES   # 62500
P = 125                       # partitions used
FB = 500                      # boxes per partition row (125*500 = 62500)
SW = 2 * FB                   # stacked width (both halves)
PI_2 = 1.5707963267948966
SC = 1.0 / 16.0               # global geometry scale (power of 2, exact)
CL = 30000.0                  # fp16-safe clamp for reciprocal planes

_CACHE = {}


def _build():
    import concourse.bass as bass
    import concourse.bacc as bacc
    import concourse.tile as tile
    from concourse import mybir

    f32 = mybir.dt.float32
    f16 = mybir.dt.float16
    AF = mybir.ActivationFunctionType
    OP = mybir.AluOpType
    AXL = mybir.AxisListType
    import os
    debug = bool(os.environ.get("K_DEBUG"))
    nc = bacc.Bacc(None, target_bir_lowering=False)
    ang_d = nc.declare_dram_parameter("ang", [2, N_CORE], f16, isOutput=False)
    wh_d = nc.declare_dram_parameter("wh", [4, N_CORE], f16, isOutput=False)
    xy_d = nc.declare_dram_parameter("xy", [4, N_CORE], f32, isOutput=False)
    out_d = nc.declare_dram_parameter("out", [P, 1], f32, isOutput=True)
    dbg_d = None
    if debug:
        dbg_d = nc.declare_dram_parameter("dbg", [4, P, FB], f32, isOutput=True)

    V = nc.vector
    S = nc.scalar

    def vtt(out, a, b, op):
        V.tensor_tensor(out, a, b, op)

    def vts(out, in_, s1, s2, op0, op1=None):
        if op1 is None:
            V.tensor_scalar(out, in_, s1, None, op0)
        else:
            V.tensor_scalar(out, in_, s1, s2, op0, op1)

    def bce(apv, n=2, axis=1):
        # stride-0 broadcast: insert a [0, n] dim at `axis` (after partition)
        ap_l = [list(d) for d in apv.ap]
        ap_l.insert(axis, [0, n])
        return bass.AP(apv.tensor, apv.offset, ap_l)

    from contextlib import ExitStack

    with tile.TileContext(nc) as tc:
        with (
            tc.tile_pool(name="pre", bufs=1) as pre,
            tc.tile_pool(name="small", bufs=1) as sm,
            ExitStack() as stack,
        ):
            io = stack.enter_context(tc.tile_pool(name="io", bufs=1))
            angT = io.tile([P, 2 * FB], f16, tag="angT")
            whT = io.tile([P, 4 * FB], f16, tag="whT")
            xyT = io.tile([P, 4 * FB], f32, tag="xyT")
            pio2 = sm.tile([P, 1], f32, tag="pio2")
            V.memset(pio2[:], PI_2)
            # 1-elem warm-up: loads the Sin ACT table while the DMA runs
            warm = sm.tile([P, 1], f32, tag="warm")
            S.activation(warm[:], pio2[:], AF.Sin)
            angV = angT[:].rearrange("p (h f) -> p h f", h=2)
            whV = whT[:].rearrange("p (c f) -> p c f", c=4)   # w1,w2,h1,h2
            xyV = xyT[:].rearrange("p (c f) -> p c f", c=4)   # x1,x2,y1,y2
            # angles first (small, unblocks the Sin chain), then wh, then xy
            nc.sync.dma_start(out=angV, in_=ang_d.rearrange("h (p f) -> p h f", p=P))
            nc.sync.dma_start(out=whV, in_=wh_d.rearrange("c (p f) -> p c f", p=P))
            nc.sync.dma_start(out=xyV, in_=xy_d.rearrange("c (p f) -> p c f", p=P))

            class SP:
                def __init__(self, name, dt=f16, w=FB, k=2):
                    self.w = w
                    self.t = pre.tile([P, k * w], dt, tag=name)

                def full(self):
                    return self.t[:]

                def h(self, i):
                    return self.t[:, i * self.w:(i + 1) * self.w]

                def v3(self):     # (P, 2, w) stacked view
                    return self.t[:].rearrange("p (h f) -> p h f", h=2)

            # paired tiles (P, 2, SW): two SW-wide planes side by side
            ddS = SP("ddS", w=SW)       # [ddx | ddy]
            cdsd = SP("cdsd", w=SW)     # [cd | sd]
            wcws = SP("wcws", w=SW)     # [wc | ws]
            hchs = SP("hchs", w=SW)     # [hc | hs]
            aP1, aP2 = SP("aP1", w=SW), SP("aP2", w=SW)
            dx16, dy16 = SP("dx16"), SP("dy16")
            dlt, dltw = SP("dlt", f32), SP("dltw", f32)
            cS, sS = SP("cS"), SP("sS")
            csS, ssS = SP("csS"), SP("ssS")
            dX, dY = SP("dX"), SP("dY")
            dXm, dYm = SP("dXm"), SP("dYm")
            whS, hhS = SP("whS"), SP("hhS")
            g0x, g0y, n1, n2 = SP("g0x"), SP("g0y"), SP("n1"), SP("n2")
            Wc, Hc, nWc, nHc = SP("Wc"), SP("Hc"), SP("nWc"), SP("nHc")
            exP, eyP = SP("exP"), SP("eyP")
            rp32a, rp32b = SP("rp32a", f32), SP("rp32b", f32)
            ddxS, ddyS = ddS.v3()[:, 0], ddS.v3()[:, 1]     # (P, SW) each
            cdS, sdS = cdsd.v3()[:, 0], cdsd.v3()[:, 1]
            wcF, wsF = wcws.v3()[:, 0], wcws.v3()[:, 1]
            hcF, hsF = hchs.v3()[:, 0], hchs.v3()[:, 1]

            def hviews(flat):     # (P, 2, FB) of an (P, SW) flat view
                return flat.rearrange("p (h f) -> p h f", h=2)

            # persistent pre-signed clamped reciprocal planes, (P, 2e, 2h, FB)
            rIX = pre.tile([P, 2 * SW], f16, tag="rIX")
            rIY = pre.tile([P, 2 * SW], f16, tag="rIY")
            rIXe = rIX[:].rearrange("p (e h f) -> p e h f", e=2, h=2)
            rIYe = rIY[:].rearrange("p (e h f) -> p e h f", e=2, h=2)

            # ---- pre-pass, angle part (only needs angT) ----
            vtt(dlt.h(0), angV[:, 0], angV[:, 1], OP.subtract)    # a1-a2 (f32)
            vts(dlt.h(1), dlt.h(0), -1.0, None, OP.mult)
            S.activation(cS.h(0), angV[:, 1], AF.Sin, bias=pio2[:])  # c2
            S.activation(cS.h(1), angV[:, 0], AF.Sin, bias=pio2[:])  # c1
            S.activation(sS.h(0), angV[:, 1], AF.Sin)                # s2
            S.activation(sS.h(1), angV[:, 0], AF.Sin)                # s1
            S.activation(sdS, dlt.full(), AF.Sin)                    # [sd|-sd]
            # cos(dlt) = sin(dlt + pi/2); wrap into [-pi, pi] first
            V.add_range_wrap(dltw.full(), dlt.full(), PI_2, 3.141592653589793,
                             6.283185307179586)
            S.activation(cdS, dltw.full(), AF.Sin)                   # [cd|cd]
            # 1/16-scaled trig copies carry the geometry scale into dX/dY
            vts(csS.full(), cS.full(), SC, None, OP.mult)
            vts(ssS.full(), sS.full(), SC, None, OP.mult)

            # ---- pre-pass, wh part ----
            vts(whS.full(), whV[:, 0:2], 0.5 * SC, None, OP.mult)  # [w1|w2]/32
            vts(hhS.full(), whV[:, 2:4], 0.5 * SC, None, OP.mult)
            # [wc|ws] = whS * [cd|sd];  [hc|hs] = hhS * [cd|sd]
            cdsd4 = cdsd.t[:].rearrange("p (c h f) -> p c h f", c=2, h=2)
            vtt(wcws.t[:].rearrange("p (c h f) -> p c h f", c=2, h=2),
                bce(whS.v3()), cdsd4, OP.mult)
            vtt(hchs.t[:].rearrange("p (c h f) -> p c h f", c=2, h=2),
                bce(hhS.v3()), cdsd4, OP.mult)
            vtt(g0x.full(), wcF, hsF, OP.subtract)
            vtt(g0y.full(), wsF, hcF, OP.add)
            vtt(n1.full(), wcF, hsF, OP.add)              # -g1x
            vtt(n2.full(), hcF, wsF, OP.subtract)         # g1y
            # clip half-extents of the fixed box, /16 (+neg)
            vts(Wc.h(0), whV[:, 1], 0.5 * SC, None, OP.mult)
            vts(Wc.h(1), whV[:, 0], 0.5 * SC, None, OP.mult)
            vts(Hc.h(0), whV[:, 3], 0.5 * SC, None, OP.mult)
            vts(Hc.h(1), whV[:, 2], 0.5 * SC, None, OP.mult)
            vts(nWc.full(), Wc.full(), -1.0, None, OP.mult)
            vts(nHc.full(), Hc.full(), -1.0, None, OP.mult)
            # moving-box bbox half-extents: ex = |wc|+|hs|, ey = |ws|+|hc|
            S.activation(aP1.full(), wcws.full(), AF.Abs)   # [|wc| | |ws|]
            S.activation(aP2.full(), hchs.full(), AF.Abs)   # [|hc| | |hs|]
            vtt(exP.full(), aP1.v3()[:, 0], aP2.v3()[:, 1], OP.add)
            vtt(eyP.full(), aP1.v3()[:, 1], aP2.v3()[:, 0], OP.add)
            # pre-signed reciprocal slab planes: rIX e0 = -1/(2wc),
            # e1 = +1/(2hs); rIY e0 = -1/(2ws), e1 = -1/(2hc).
            # clamp to +-CL then fp16 so 0*inf NaNs cannot occur.
            for (dst, src, sgn, rp) in (
                (rIXe[:, 0], wcF, -1.0, rp32a),
                (rIXe[:, 1], hsF, 1.0, rp32b),
                (rIYe[:, 0], wsF, -1.0, rp32a),
                (rIYe[:, 1], hcF, -1.0, rp32b),
            ):
                vts(rp.full(), src, 2.0 * sgn, 1e-20 * sgn, OP.mult, OP.add)
                V.reciprocal_approx_fast(out=rp.full(), in_=rp.full())
                vts(dst, rp.v3(), CL, -CL, OP.min, OP.max)
            # union0 = (w1h1 + w2h2)/1024; the *4 to reach the /256 scale of
            # inter is folded into the final union STT
            u01 = sm.tile([P, SW], f16, tag="u01")
            union0 = sm.tile([P, FB], f32, tag="union0")
            vtt(u01[:], whS.full(), hhS.full(), OP.mult)
            u013 = u01[:].rearrange("p (h f) -> p h f", h=2)
            vtt(union0[:], u013[:, 0], u013[:, 1], OP.add)

            # ---- pre-pass, xy part (lands last) ----
            vtt(hviews(ddxS)[:, 0], xyV[:, 0], xyV[:, 1], OP.subtract)  # x1-x2
            vts(hviews(ddxS)[:, 1], hviews(ddxS)[:, 0], -1.0, None, OP.mult)
            vtt(hviews(ddyS)[:, 0], xyV[:, 2], xyV[:, 3], OP.subtract)
            vts(hviews(ddyS)[:, 1], hviews(ddyS)[:, 0], -1.0, None, OP.mult)
            vts(dx16.full(), ddxS, SC, None, OP.mult)
            vts(dy16.full(), ddyS, SC, None, OP.mult)
            # delta = R^T * (center difference)/16, stacked:
            # P1 = [csS*ddx | csS*ddy], P2 = [ssS*ddx | ssS*ddy]
            ddc = ddS.t[:].rearrange("p (c h f) -> p c h f", c=2, h=2)
            vtt(aP1.t[:].rearrange("p (c h f) -> p c h f", c=2, h=2),
                bce(csS.v3()), ddc, OP.mult)
            vtt(aP2.t[:].rearrange("p (c h f) -> p c h f", c=2, h=2),
                bce(ssS.v3()), ddc, OP.mult)
            vtt(dX.full(), aP1.v3()[:, 0], aP2.v3()[:, 1], OP.add)
            vtt(dY.full(), aP1.v3()[:, 1], aP2.v3()[:, 0], OP.subtract)
            vts(dXm.full(), dX.full(), 2.0, None, OP.mult)        # 2*dx
            vts(dYm.full(), dY.full(), 2.0, None, OP.mult)

            # input tiles no longer needed: free the io pool
            stack.close()
            hv = stack.enter_context(tc.tile_pool(name="heavy", bufs=1))

            def E(tile4):     # (P, 4, 2, FB) edge/half view of 4*SW tile
                return tile4[:].rearrange("p (e h f) -> p e h f", e=4, h=2)

            AXt = hv.tile([P, 4 * SW], f16, tag="AXt")
            AYt = hv.tile([P, 4 * SW], f16, tag="AYt")
            DRX = hv.tile([P, 4 * SW], f16, tag="DRX")
            DRY = hv.tile([P, 4 * SW], f16, tag="DRY")
            Ut = hv.tile([P, 4 * SW], f16, tag="Ut")
            Vt = hv.tile([P, 4 * SW], f16, tag="Vt")
            NPt = hv.tile([P, 4 * SW], f16, tag="NPt")
            TLX = hv.tile([P, 4 * SW], f16, tag="TLX")

            # corners: e0,e1 explicit; e2,e3 = 2*dX - e0,e1 (point symmetry)
            vtt(E(AXt)[:, 0], dX.v3(), g0x.v3(), OP.add)
            vtt(E(AXt)[:, 1], dX.v3(), n1.v3(), OP.subtract)
            vtt(E(AXt)[:, 2:4], bce(dXm.v3()), E(AXt)[:, 0:2], OP.subtract)
            vtt(E(AYt)[:, 0], dY.v3(), g0y.v3(), OP.add)
            vtt(E(AYt)[:, 1], dY.v3(), n2.v3(), OP.add)
            vtt(E(AYt)[:, 2:4], bce(dYm.v3()), E(AYt)[:, 0:2], OP.subtract)

            # ---- enclosing rect (bbox in each frame, min of the two) ----
            exm = sm.tile([P, SW], f16, tag="exm")
            exn = sm.tile([P, SW], f16, tag="exn")
            exs = sm.tile([P, SW], f16, tag="exs")
            eys = sm.tile([P, SW], f16, tag="eys")
            ex3 = exm[:].rearrange("p (h f) -> p h f", h=2)
            en3 = exn[:].rearrange("p (h f) -> p h f", h=2)
            es3 = exs[:].rearrange("p (h f) -> p h f", h=2)
            ey3 = eys[:].rearrange("p (h f) -> p h f", h=2)
            for ext, d3, clamp, dst3 in ((exP, dX, Wc, es3), (eyP, dY, Hc, ey3)):
                vtt(ex3, d3.v3(), ext.v3(), OP.add)               # dX + ex
                vtt(en3, ext.v3(), d3.v3(), OP.subtract)          # ex - dX
                vtt(ex3, ex3, clamp.v3(), OP.max)
                vtt(en3, en3, clamp.v3(), OP.max)
                vtt(dst3, ex3, en3, OP.add)                       # extent
            vtt(exs[:], exs[:], eys[:], OP.mult)                  # areaC stacked
            area_c = sm.tile([P, FB], f32, tag="area_c")
            vtt(area_c[:], es3[:, 0], es3[:, 1], OP.min)

            HW2 = 2 * SW

            def H01(t4):
                return t4[:, 0:HW2]

            def H23(t4):
                return t4[:, HW2:2 * HW2]

            # ---- slab clip, x axis, edges 0,1 (2,3 via point symmetry:
            # roots(edge2) = m + roots(edge0), m = dXm*rIX) ----
            vtt(E(Ut)[:, 0:2], bce(nWc.v3()), E(AXt)[:, 0:2], OP.subtract)
            vtt(E(Vt)[:, 0:2], bce(Wc.v3()), E(AXt)[:, 0:2], OP.subtract)
            vtt(H01(Ut), H01(Ut), rIX[:], OP.mult)                 # ta01
            vtt(H01(Vt), H01(Vt), rIX[:], OP.mult)                 # tb01
            vtt(H01(TLX), H01(Ut), H01(Vt), OP.min)                # tlo01
            vtt(H01(Ut), H01(Ut), H01(Vt), OP.max)                 # thi01
            vtt(E(Vt)[:, 0:2], bce(dXm.v3()), rIXe, OP.mult)       # m01
            vtt(H23(TLX), H01(Vt), H01(TLX), OP.add)               # tlo23
            vtt(H23(Ut), H01(Vt), H01(Ut), OP.add)                 # thi23
            # ---- slab clip, y axis, edges 0,1 ----
            vtt(E(Vt)[:, 0:2], bce(nHc.v3()), E(AYt)[:, 0:2], OP.subtract)
            vtt(E(NPt)[:, 0:2], bce(Hc.v3()), E(AYt)[:, 0:2], OP.subtract)
            vtt(H01(Vt), H01(Vt), rIY[:], OP.mult)                 # ta01_y
            vtt(H01(NPt), H01(NPt), rIY[:], OP.mult)               # tb01_y
            vtt(H01(DRX), H01(Vt), H01(NPt), OP.min)               # tlo01_y
            vtt(H01(Vt), H01(Vt), H01(NPt), OP.max)                # thi01_y
            vtt(E(NPt)[:, 0:2], bce(dYm.v3()), rIYe, OP.mult)      # m01_y
            vtt(H23(DRX), H01(NPt), H01(DRX), OP.add)              # tlo23_y
            vtt(H23(Vt), H01(NPt), H01(Vt), OP.add)                # thi23_y
            # ---- interval intersect, dt ----
            # t0 = max(tlo_x, tlo_y, 0); t1 = min(thi_x, thi_y, 1)
            vtt(TLX[:], TLX[:], DRX[:], OP.max)
            vts(TLX[:], TLX[:], 0.0, None, OP.max)
            vtt(Ut[:], Ut[:], Vt[:], OP.min)
            vts(Ut[:], Ut[:], 1.0, None, OP.min)
            vtt(TLX[:], Ut[:], TLX[:], OP.subtract)                # t1-t0
            S.activation(TLX[:], TLX[:], AF.Relu)                  # dt
            # ---- direction planes (on Scalar), cross(a,d), pieces ----
            for dst, srcs in (
                (DRX, ((wcF, -2.0), (hsF, 2.0), (wcF, 2.0), (hsF, -2.0))),
                (DRY, ((wsF, -2.0), (hcF, -2.0), (wsF, 2.0), (hcF, 2.0))),
            ):
                d4 = E(dst)
                for e, (src, sc) in enumerate(srcs):
                    S.activation(d4[:, e], hviews(src), AF.Copy, scale=sc)
            vtt(Vt[:], AXt[:], DRY[:], OP.mult)                    # ax*dy
            vtt(NPt[:], AYt[:], DRX[:], OP.mult)                   # ay*dx
            vtt(Vt[:], Vt[:], NPt[:], OP.subtract)                 # cad
            vtt(Ut[:], TLX[:], Vt[:], OP.mult)                     # pieces

            # ---- piece sum (stacked), SA correction (frame-B half) ----
            psS = sm.tile([P, SW], f16, tag="psS")
            ps3 = psS[:].rearrange("p (h f) -> p h f", h=2)
            u4 = E(Ut)
            vtt(ps3, u4[:, 0], u4[:, 1], OP.add)
            vtt(es3, u4[:, 2], u4[:, 3], OP.add)                   # reuse exs
            vtt(ps3, ps3, es3, OP.add)
            dt4 = E(TLX)
            sax = sm.tile([P, FB], f16, tag="sax")
            say = sm.tile([P, FB], f16, tag="say")
            sau = sm.tile([P, FB], f16, tag="sau")
            sav = sm.tile([P, FB], f16, tag="sav")
            st1 = sm.tile([P, FB], f16, tag="st1")
            vtt(sau[:], dt4[:, 2, 0], dt4[:, 0, 0], OP.subtract)
            vtt(sav[:], dt4[:, 3, 0], dt4[:, 1, 0], OP.subtract)
            dx4, dy4 = E(DRX), E(DRY)
            vtt(sax[:], dx4[:, 2, 0], sau[:], OP.mult)
            vtt(st1[:], dx4[:, 3, 0], sav[:], OP.mult)
            vtt(sax[:], sax[:], st1[:], OP.add)
            vtt(say[:], dy4[:, 2, 0], sau[:], OP.mult)
            vtt(st1[:], dy4[:, 3, 0], sav[:], OP.mult)
            vtt(say[:], say[:], st1[:], OP.add)
            # corr = ddy/16*(c2*sax - s2*say) - ddx/16*(s2*sax + c2*say)
            c2v = cS.h(0)
            s2v = sS.h(0)
            rsx = sm.tile([P, FB], f16, tag="rsx")
            rsy = sm.tile([P, FB], f16, tag="rsy")
            vtt(rsx[:], c2v, sax[:], OP.mult)
            vtt(st1[:], s2v, say[:], OP.mult)
            vtt(rsx[:], rsx[:], st1[:], OP.subtract)
            vtt(rsy[:], s2v, sax[:], OP.mult)
            vtt(st1[:], c2v, say[:], OP.mult)
            vtt(rsy[:], rsy[:], st1[:], OP.add)
            inter16 = sm.tile([P, FB], f16, tag="inter16")
            vtt(inter16[:], dy16.h(0), rsx[:], OP.mult)
            vtt(st1[:], dx16.h(0), rsy[:], OP.mult)
            vtt(inter16[:], inter16[:], st1[:], OP.subtract)       # corr
            vtt(inter16[:], inter16[:], ps3[:, 0], OP.add)
            vtt(inter16[:], inter16[:], ps3[:, 1], OP.add)
            inter = sm.tile([P, FB], f32, tag="inter")
            S.activation(inter[:], inter16[:], AF.Relu, scale=0.5)  # inter area

            # ---- final loss (fp32) ----
            union = sm.tile([P, FB], f32, tag="union")
            fr1 = sm.tile([P, FB], f32, tag="fr1")
            iou = sm.tile([P, FB], f32, tag="iou")
            rr = sm.tile([P, FB], f32, tag="rr")
            lsa = sm.tile([P, 1], f32, tag="lsa")
            # union = 4*union0 - inter  (the *4 restores the /256 scale)
            V.scalar_tensor_tensor(union[:], union0[:], 4.0, inter[:],
                                   OP.mult, OP.subtract)
            V.reciprocal_approx_fast(out=fr1[:], in_=union[:])
            vtt(iou[:], inter[:], fr1[:], OP.mult)
            vts(iou[:], iou[:], 1e-6, None, OP.max)
            V.reciprocal_approx_fast(out=fr1[:], in_=area_c[:])
            vtt(fr1[:], union[:], fr1[:], OP.mult)
            vts(rr[:], fr1[:], -1.0, 1.0, OP.mult, OP.add)         # 1 - u/ac
            vtt(fr1[:], iou[:], iou[:], OP.mult)                   # iou^2
            vtt(fr1[:], fr1[:], iou[:], OP.mult)                   # iou^3
            vtt(iou[:], rr[:], rr[:], OP.mult)                     # rr^2
            vtt(iou[:], iou[:], rr[:], OP.mult)                    # rr^3
            vtt(fr1[:], fr1[:], iou[:], OP.subtract)               # giou
            V.tensor_reduce(lsa[:], fr1[:], AXL.X, OP.add)         # sum giou
            if debug:
                nc.sync.dma_start(out=dbg_d[0], in_=fr1[:])
                nc.sync.dma_start(out=dbg_d[1], in_=inter[:])
                nc.sync.dma_start(out=dbg_d[2], in_=union[:])
                nc.sync.dma_start(out=dbg_d[3], in_=area_c[:])
            nc.sync.dma_start(out=out_d[:], in_=lsa[:])

    nc.finalize()
    return nc


def _get_nc():
    if "nc" not in _CACHE:
        _CACHE["nc"] = _build()
    return _CACHE["nc"]


def _repack(pred, target):
    """Per-core input repack: planar rows so every SBUF slice is packed.
    ang/wh in fp16 (small, fast DMA); xy in fp32 (exact center diffs)."""
    in_maps = []
    for i in range(N_CORES):
        sl = slice(i * N_CORE, (i + 1) * N_CORE)
        p, t = pred[sl], target[sl]
        ang = np.stack([p[:, 4], t[:, 4]]).astype(np.float16)
        wh = np.stack([p[:, 2], t[:, 2], p[:, 3], t[:, 3]]).astype(np.float16)
        xy = np.ascontiguousarray(np.stack([p[:, 0], t[:, 0], p[:, 1], t[:, 1]]))
        in_maps.append({"ang": ang, "wh": wh, "xy": xy})
    return in_maps


def kernel(pred, target):
    from concourse.bass_utils import run_bass_kernel_spmd

    pred = np.ascontiguousarray(np.asarray(pred, dtype=np.float32))
    target = np.ascontiguousarray(np.asarray(target, dtype=np.float32))
    nc = _get_nc()
    in_maps = _repack(pred, target)
    res = run_bass_kernel_spmd(nc, in_maps, core_ids=list(range(N_CORES)))
    gsum = np.float64(0.0)
    for i in range(N_CORES):
        gsum += np.asarray(res.results[i]["out"], dtype=np.float64).sum()
    # loss = mean(1 - giou) = 1 - sum(giou)/N
    return np.float32(1.0 - gsum / N_TOTAL)


# revision 17
# speedup vs baseline: 1.7494x; 1.0447x over previous
"""AlphaRotatedGIoULoss on 8 TRN2 NeuronCores.

Data-parallel: 500000 box pairs sharded 62500/core, laid out as
(125 partitions x 500 boxes). Per-box rotated-GIoU via a branchless
line-integral intersection (slab clipping in each box's axis-aligned
frame + a frame-change correction term), so no sorting/gather is needed.

v4: heavy elementwise chain in fp16 (DVE 2x_1p mode = 2x throughput),
geometry pre-scaled by 1/16 (folded into existing scale factors) so all
products stay in fp16 range; reciprocal slab planes clamped to +-3e4 so
0*inf NaNs cannot occur. Host repack: angle and w/h columns are cast to
fp16 (error ~0.06px, far under tolerance) and shipped as planar rows so
every SBUF slice is packed; xy stays fp32 for exact center differences.
DMA order ang -> wh -> xy unblocks the Sin chain ~2us in. Point-symmetry
(corner e2,e3 = 2*dX - e0,e1) is exploited with stride-0 broadcast APs
to merge op pairs into single wide DVE passes. Enclosing-box extents
use half_extent_x = |wc|+|hs|. Output: per-core sum(giou); host 1-s/N.
"""
import sys
import numpy as np

for _p in ("/opt/trn_rl_repo", "/root/.axon_site/_ro/trn_rl_repo"):
    if _p not in sys.path:
        sys.path.insert(0, _p)

N_CORES = 8
N_TOTAL = 500000
N_CORE = N_TOTAL // N_CORES   # 62500
P = 128                       # all partitions
FB = 489                      # boxes per partition row (128*489 = 62592)
NPAD = P * FB                 # per-core padded count (92 identity pad boxes)
SW = 2 * FB                   # stacked width (both halves)
PI_2 = 1.5707963267948966
SC = 1.0 / 16.0               # global geometry scale (power of 2, exact)
XQ = 32.0                     # xy fixed-point scale (int16 units = px/32)
XSC = SC / XQ                 # folds the xy dequant into the trig scale
CL = 30000.0                  # fp16-safe clamp for reciprocal planes

_CACHE = {}


def _build():
    import concourse.bass as bass
    import concourse.bacc as bacc
    import concourse.tile as tile
    from concourse import mybir

    f32 = mybir.dt.float32
    f16 = mybir.dt.float16
    i16 = mybir.dt.int16
    AF = mybir.ActivationFunctionType
    OP = mybir.AluOpType
    AXL = mybir.AxisListType
    import os
    debug = bool(os.environ.get("K_DEBUG"))
    nc = bacc.Bacc(None, target_bir_lowering=False)
    ang_d = nc.declare_dram_parameter("ang", [2, NPAD], f16, isOutput=False)
    wh_d = nc.declare_dram_parameter("wh", [4, NPAD], f16, isOutput=False)
    xy_d = nc.declare_dram_parameter("xy", [4, NPAD], i16, isOutput=False)
    out_d = nc.declare_dram_parameter("out", [P, 1], f32, isOutput=True)
    dbg_d = None
    if debug:
        dbg_d = nc.declare_dram_parameter("dbg", [4, P, FB], f32, isOutput=True)

    V = nc.vector
    S = nc.scalar

    def vtt(out, a, b, op):
        V.tensor_tensor(out, a, b, op)

    def vts(out, in_, s1, s2, op0, op1=None):
        if op1 is None:
            V.tensor_scalar(out, in_, s1, None, op0)
        else:
            V.tensor_scalar(out, in_, s1, s2, op0, op1)

    def bce(apv, n=2, axis=1):
        # stride-0 broadcast: insert a [0, n] dim at `axis` (after partition)
        ap_l = [list(d) for d in apv.ap]
        ap_l.insert(axis, [0, n])
        return bass.AP(apv.tensor, apv.offset, ap_l)

    from contextlib import ExitStack

    with tile.TileContext(nc) as tc:
        with (
            tc.tile_pool(name="pre", bufs=1) as pre,
            tc.tile_pool(name="small", bufs=1) as sm,
            ExitStack() as stack,
        ):
            io = stack.enter_context(tc.tile_pool(name="io", bufs=1))
            angT = io.tile([P, 2 * FB], f16, tag="angT")
            whT = io.tile([P, 4 * FB], f16, tag="whT")
            xyT = io.tile([P, 4 * FB], i16, tag="xyT")
            pio2 = sm.tile([P, 1], f32, tag="pio2")
            V.memset(pio2[:], PI_2)
            # 1-elem warm-up: loads the Sin ACT table while the DMA runs
            warm = sm.tile([P, 1], f32, tag="warm")
            S.activation(warm[:], pio2[:], AF.Sin)
            angV = angT[:].rearrange("p (h f) -> p h f", h=2)
            whV = whT[:].rearrange("p (c f) -> p c f", c=4)   # w1,w2,h1,h2
            xyV = xyT[:].rearrange("p (c f) -> p c f", c=4)   # x1,x2,y1,y2
            # angles first (small, unblocks the Sin chain), then wh, then xy
            nc.sync.dma_start(out=angV, in_=ang_d.rearrange("h (p f) -> p h f", p=P))
            nc.sync.dma_start(out=whV, in_=wh_d.rearrange("c (p f) -> p c f", p=P))
            nc.sync.dma_start(out=xyV, in_=xy_d.rearrange("c (p f) -> p c f", p=P))

            class SP:
                def __init__(self, name, dt=f16, w=FB, k=2):
                    self.w = w
                    self.t = pre.tile([P, k * w], dt, tag=name)

                def full(self):
                    return self.t[:]

                def h(self, i):
                    return self.t[:, i * self.w:(i + 1) * self.w]

                def v3(self):     # (P, 2, w) stacked view
                    return self.t[:].rearrange("p (h f) -> p h f", h=2)

            # paired tiles (P, 2, SW): two SW-wide planes side by side
            ddS = SP("ddS", w=SW)       # [ddx | ddy]
            cdsd = SP("cdsd", w=SW)     # [cd | sd]
            wcws = SP("wcws", w=SW)     # [wc | ws]
            hchs = SP("hchs", w=SW)     # [hc | hs]
            aP1, aP2 = SP("aP1", w=SW), SP("aP2", w=SW)
            dx16, dy16 = SP("dx16"), SP("dy16")
            dlt, dltw = SP("dlt", f32), SP("dltw", f32)
            cS, sS = SP("cS"), SP("sS")
            csS, ssS = SP("csS"), SP("ssS")
            dX, dY = SP("dX"), SP("dY")
            dXm, dYm = SP("dXm"), SP("dYm")
            whS, hhS = SP("whS"), SP("hhS")
            g0x, g0y, n1, n2 = SP("g0x"), SP("g0y"), SP("n1"), SP("n2")
            Wc, Hc, nWc, nHc = SP("Wc"), SP("Hc"), SP("nWc"), SP("nHc")
            exP, eyP = SP("exP"), SP("eyP")
            rp32a, rp32b = SP("rp32a", f32), SP("rp32b", f32)
            ddxS, ddyS = ddS.v3()[:, 0], ddS.v3()[:, 1]     # (P, SW) each
            cdS, sdS = cdsd.v3()[:, 0], cdsd.v3()[:, 1]
            wcF, wsF = wcws.v3()[:, 0], wcws.v3()[:, 1]
            hcF, hsF = hchs.v3()[:, 0], hchs.v3()[:, 1]

            def hviews(flat):     # (P, 2, FB) of an (P, SW) flat view
                return flat.rearrange("p (h f) -> p h f", h=2)

            # persistent pre-signed clamped reciprocal planes, (P, 2e, 2h, FB)
            rIX = pre.tile([P, 2 * SW], f16, tag="rIX")
            rIY = pre.tile([P, 2 * SW], f16, tag="rIY")
            rIXe = rIX[:].rearrange("p (e h f) -> p e h f", e=2, h=2)
            rIYe = rIY[:].rearrange("p (e h f) -> p e h f", e=2, h=2)

            # ---- pre-pass, angle part (only needs angT) ----
            vtt(dlt.h(0), angV[:, 0], angV[:, 1], OP.subtract)    # a1-a2 (f32)
            vts(dlt.h(1), dlt.h(0), -1.0, None, OP.mult)
            S.activation(cS.h(0), angV[:, 1], AF.Sin, bias=pio2[:])  # c2
            S.activation(cS.h(1), angV[:, 0], AF.Sin, bias=pio2[:])  # c1
            S.activation(sS.h(0), angV[:, 1], AF.Sin)                # s2
            S.activation(sS.h(1), angV[:, 0], AF.Sin)                # s1
            S.activation(sdS, dlt.full(), AF.Sin)                    # [sd|-sd]
            # cos(dlt) = sin(dlt + pi/2); wrap into [-pi, pi] first
            V.add_range_wrap(dltw.full(), dlt.full(), PI_2, 3.141592653589793,
                             6.283185307179586)
            S.activation(cdS, dltw.full(), AF.Sin)                   # [cd|cd]
            # scaled trig copies carry geometry scale + xy dequant into dX/dY
            vts(csS.full(), cS.full(), XSC, None, OP.mult)
            vts(ssS.full(), sS.full(), XSC, None, OP.mult)

            # ---- pre-pass, wh part ----
            vts(whS.full(), whV[:, 0:2], 0.5 * SC, None, OP.mult)  # [w1|w2]/32
            vts(hhS.full(), whV[:, 2:4], 0.5 * SC, None, OP.mult)
            # [wc|ws] = whS * [cd|sd];  [hc|hs] = hhS * [cd|sd]
            cdsd4 = cdsd.t[:].rearrange("p (c h f) -> p c h f", c=2, h=2)
            vtt(wcws.t[:].rearrange("p (c h f) -> p c h f", c=2, h=2),
                bce(whS.v3()), cdsd4, OP.mult)
            vtt(hchs.t[:].rearrange("p (c h f) -> p c h f", c=2, h=2),
                bce(hhS.v3()), cdsd4, OP.mult)
            vtt(g0x.full(), wcF, hsF, OP.subtract)
            vtt(g0y.full(), wsF, hcF, OP.add)
            vtt(n1.full(), wcF, hsF, OP.add)              # -g1x
            vtt(n2.full(), hcF, wsF, OP.subtract)         # g1y
            # clip half-extents of the fixed box, /16 (+neg)
            vts(Wc.h(0), whV[:, 1], 0.5 * SC, None, OP.mult)
            vts(Wc.h(1), whV[:, 0], 0.5 * SC, None, OP.mult)
            vts(Hc.h(0), whV[:, 3], 0.5 * SC, None, OP.mult)
            vts(Hc.h(1), whV[:, 2], 0.5 * SC, None, OP.mult)
            vts(nWc.full(), Wc.full(), -1.0, None, OP.mult)
            vts(nHc.full(), Hc.full(), -1.0, None, OP.mult)
            # moving-box bbox half-extents: ex = |wc|+|hs|, ey = |ws|+|hc|
            S.activation(aP1.full(), wcws.full(), AF.Abs)   # [|wc| | |ws|]
            S.activation(aP2.full(), hchs.full(), AF.Abs)   # [|hc| | |hs|]
            vtt(exP.full(), aP1.v3()[:, 0], aP2.v3()[:, 1], OP.add)
            vtt(eyP.full(), aP1.v3()[:, 1], aP2.v3()[:, 0], OP.add)
            # pre-signed reciprocal slab planes: rIX e0 = -1/(2wc),
            # e1 = +1/(2hs); rIY e0 = -1/(2ws), e1 = -1/(2hc).
            # clamp to +-CL then fp16 so 0*inf NaNs cannot occur.
            for (dst, src, sgn, rp) in (
                (rIXe[:, 0], wcF, -1.0, rp32a),
                (rIXe[:, 1], hsF, 1.0, rp32b),
                (rIYe[:, 0], wsF, -1.0, rp32a),
                (rIYe[:, 1], hcF, -1.0, rp32b),
            ):
                vts(rp.full(), src, 2.0 * sgn, 1e-20 * sgn, OP.mult, OP.add)
                V.reciprocal_approx_fast(out=rp.full(), in_=rp.full())
                vts(dst, rp.v3(), CL, -CL, OP.min, OP.max)
            # union0 = (w1h1 + w2h2)/1024; the *4 to reach the /256 scale of
            # inter is folded into the final union STT
            u01 = sm.tile([P, SW], f16, tag="u01")
            union0 = sm.tile([P, FB], f32, tag="union0")
            vtt(u01[:], whS.full(), hhS.full(), OP.mult)
            u013 = u01[:].rearrange("p (h f) -> p h f", h=2)
            vtt(union0[:], u013[:, 0], u013[:, 1], OP.add)

            # ---- pre-pass, xy part (lands last) ----
            vtt(hviews(ddxS)[:, 0], xyV[:, 0], xyV[:, 1], OP.subtract)  # x1-x2
            vts(hviews(ddxS)[:, 1], hviews(ddxS)[:, 0], -1.0, None, OP.mult)
            vtt(hviews(ddyS)[:, 0], xyV[:, 2], xyV[:, 3], OP.subtract)
            vts(hviews(ddyS)[:, 1], hviews(ddyS)[:, 0], -1.0, None, OP.mult)
            vts(dx16.full(), ddxS, XSC, None, OP.mult)
            vts(dy16.full(), ddyS, XSC, None, OP.mult)
            # delta = R^T * (center difference)/16, stacked:
            # P1 = [csS*ddx | csS*ddy], P2 = [ssS*ddx | ssS*ddy]
            ddc = ddS.t[:].rearrange("p (c h f) -> p c h f", c=2, h=2)
            vtt(aP1.t[:].rearrange("p (c h f) -> p c h f", c=2, h=2),
                bce(csS.v3()), ddc, OP.mult)
            vtt(aP2.t[:].rearrange("p (c h f) -> p c h f", c=2, h=2),
                bce(ssS.v3()), ddc, OP.mult)
            vtt(dX.full(), aP1.v3()[:, 0], aP2.v3()[:, 1], OP.add)
            vtt(dY.full(), aP1.v3()[:, 1], aP2.v3()[:, 0], OP.subtract)
            vts(dXm.full(), dX.full(), 2.0, None, OP.mult)        # 2*dx
            vts(dYm.full(), dY.full(), 2.0, None, OP.mult)

            # input tiles no longer needed: free the io pool
            stack.close()
            hv = stack.enter_context(tc.tile_pool(name="heavy", bufs=1))

            def E(tile4):     # (P, 4, 2, FB) edge/half view of 4*SW tile
                return tile4[:].rearrange("p (e h f) -> p e h f", e=4, h=2)

            AXt = hv.tile([P, 4 * SW], f16, tag="AXt")
            AYt = hv.tile([P, 4 * SW], f16, tag="AYt")
            DRX = hv.tile([P, 4 * SW], f16, tag="DRX")
            DRY = hv.tile([P, 4 * SW], f16, tag="DRY")
            Ut = hv.tile([P, 4 * SW], f16, tag="Ut")
            Vt = hv.tile([P, 4 * SW], f16, tag="Vt")
            NPt = hv.tile([P, 4 * SW], f16, tag="NPt")
            TLX = hv.tile([P, 4 * SW], f16, tag="TLX")

            # corners: e0,e1 explicit; e2,e3 = 2*dX - e0,e1 (point symmetry)
            vtt(E(AXt)[:, 0], dX.v3(), g0x.v3(), OP.add)
            vtt(E(AXt)[:, 1], dX.v3(), n1.v3(), OP.subtract)
            vtt(E(AXt)[:, 2:4], bce(dXm.v3()), E(AXt)[:, 0:2], OP.subtract)
            vtt(E(AYt)[:, 0], dY.v3(), g0y.v3(), OP.add)
            vtt(E(AYt)[:, 1], dY.v3(), n2.v3(), OP.add)
            vtt(E(AYt)[:, 2:4], bce(dYm.v3()), E(AYt)[:, 0:2], OP.subtract)

            # ---- enclosing rect (bbox in each frame, min of the two) ----
            exm = sm.tile([P, SW], f16, tag="exm")
            exn = sm.tile([P, SW], f16, tag="exn")
            exs = sm.tile([P, SW], f16, tag="exs")
            eys = sm.tile([P, SW], f16, tag="eys")
            ex3 = exm[:].rearrange("p (h f) -> p h f", h=2)
            en3 = exn[:].rearrange("p (h f) -> p h f", h=2)
            es3 = exs[:].rearrange("p (h f) -> p h f", h=2)
            ey3 = eys[:].rearrange("p (h f) -> p h f", h=2)
            for ext, d3, clamp, dst3 in ((exP, dX, Wc, es3), (eyP, dY, Hc, ey3)):
                vtt(ex3, d3.v3(), ext.v3(), OP.add)               # dX + ex
                vtt(en3, ext.v3(), d3.v3(), OP.subtract)          # ex - dX
                vtt(ex3, ex3, clamp.v3(), OP.max)
                vtt(en3, en3, clamp.v3(), OP.max)
                vtt(dst3, ex3, en3, OP.add)                       # extent
            vtt(exs[:], exs[:], eys[:], OP.mult)                  # areaC stacked
            area_c = sm.tile([P, FB], f32, tag="area_c")
            vtt(area_c[:], es3[:, 0], es3[:, 1], OP.min)

            HW2 = 2 * SW

            def H01(t4):
                return t4[:, 0:HW2]

            def H23(t4):
                return t4[:, HW2:2 * HW2]

            # ---- slab clip, x axis, edges 0,1 (2,3 via point symmetry:
            # roots(edge2) = m + roots(edge0), m = dXm*rIX) ----
            vtt(E(Ut)[:, 0:2], bce(nWc.v3()), E(AXt)[:, 0:2], OP.subtract)
            vtt(E(Vt)[:, 0:2], bce(Wc.v3()), E(AXt)[:, 0:2], OP.subtract)
            vtt(H01(Ut), H01(Ut), rIX[:], OP.mult)                 # ta01
            vtt(H01(Vt), H01(Vt), rIX[:], OP.mult)                 # tb01
            vtt(H01(TLX), H01(Ut), H01(Vt), OP.min)                # tlo01
            vtt(H01(Ut), H01(Ut), H01(Vt), OP.max)                 # thi01
            vtt(E(Vt)[:, 0:2], bce(dXm.v3()), rIXe, OP.mult)       # m01
            vtt(H23(TLX), H01(Vt), H01(TLX), OP.add)               # tlo23
            vtt(H23(Ut), H01(Vt), H01(Ut), OP.add)                 # thi23
            # ---- slab clip, y axis, edges 0,1 ----
            vtt(E(Vt)[:, 0:2], bce(nHc.v3()), E(AYt)[:, 0:2], OP.subtract)
            vtt(E(NPt)[:, 0:2], bce(Hc.v3()), E(AYt)[:, 0:2], OP.subtract)
            vtt(H01(Vt), H01(Vt), rIY[:], OP.mult)                 # ta01_y
            vtt(H01(NPt), H01(NPt), rIY[:], OP.mult)               # tb01_y
            vtt(H01(DRX), H01(Vt), H01(NPt), OP.min)               # tlo01_y
            vtt(H01(Vt), H01(Vt), H01(NPt), OP.max)                # thi01_y
            vtt(E(NPt)[:, 0:2], bce(dYm.v3()), rIYe, OP.mult)      # m01_y
            vtt(H23(DRX), H01(NPt), H01(DRX), OP.add)              # tlo23_y
            vtt(H23(Vt), H01(NPt), H01(Vt), OP.add)                # thi23_y
            # ---- interval intersect, dt ----
            # t0 = max(tlo_x, tlo_y, 0); t1 = min(thi_x, thi_y, 1)
            vtt(TLX[:], TLX[:], DRX[:], OP.max)
            vts(TLX[:], TLX[:], 0.0, None, OP.max)
            vtt(Ut[:], Ut[:], Vt[:], OP.min)
            vts(Ut[:], Ut[:], 1.0, None, OP.min)
            vtt(TLX[:], Ut[:], TLX[:], OP.subtract)                # t1-t0
            S.activation(TLX[:], TLX[:], AF.Relu)                  # dt
            # ---- direction planes (on Scalar), cross(a,d), pieces ----
            for dst, srcs in (
                (DRX, ((wcF, -2.0), (hsF, 2.0), (wcF, 2.0), (hsF, -2.0))),
                (DRY, ((wsF, -2.0), (hcF, -2.0), (wsF, 2.0), (hcF, 2.0))),
            ):
                d4 = E(dst)
                for e, (src, sc) in enumerate(srcs):
                    S.activation(d4[:, e], hviews(src), AF.Copy, scale=sc)
            vtt(Vt[:], AXt[:], DRY[:], OP.mult)                    # ax*dy
            vtt(NPt[:], AYt[:], DRX[:], OP.mult)                   # ay*dx
            vtt(Vt[:], Vt[:], NPt[:], OP.subtract)                 # cad
            vtt(Ut[:], TLX[:], Vt[:], OP.mult)                     # pieces

            # ---- piece sum (stacked), SA correction (frame-B half) ----
            psS = sm.tile([P, SW], f16, tag="psS")
            ps3 = psS[:].rearrange("p (h f) -> p h f", h=2)
            u4 = E(Ut)
            vtt(ps3, u4[:, 0], u4[:, 1], OP.add)
            vtt(es3, u4[:, 2], u4[:, 3], OP.add)                   # reuse exs
            vtt(ps3, ps3, es3, OP.add)
            dt4 = E(TLX)
            sax = sm.tile([P, FB], f16, tag="sax")
            say = sm.tile([P, FB], f16, tag="say")
            sau = sm.tile([P, FB], f16, tag="sau")
            sav = sm.tile([P, FB], f16, tag="sav")
            st1 = sm.tile([P, FB], f16, tag="st1")
            vtt(sau[:], dt4[:, 2, 0], dt4[:, 0, 0], OP.subtract)
            vtt(sav[:], dt4[:, 3, 0], dt4[:, 1, 0], OP.subtract)
            dx4, dy4 = E(DRX), E(DRY)
            vtt(sax[:], dx4[:, 2, 0], sau[:], OP.mult)
            vtt(st1[:], dx4[:, 3, 0], sav[:], OP.mult)
            vtt(sax[:], sax[:], st1[:], OP.add)
            vtt(say[:], dy4[:, 2, 0], sau[:], OP.mult)
            vtt(st1[:], dy4[:, 3, 0], sav[:], OP.mult)
            vtt(say[:], say[:], st1[:], OP.add)
            # corr = ddy/16*(c2*sax - s2*say) - ddx/16*(s2*sax + c2*say)
            c2v = cS.h(0)
            s2v = sS.h(0)
            rsx = sm.tile([P, FB], f16, tag="rsx")
            rsy = sm.tile([P, FB], f16, tag="rsy")
            vtt(rsx[:], c2v, sax[:], OP.mult)
            vtt(st1[:], s2v, say[:], OP.mult)
            vtt(rsx[:], rsx[:], st1[:], OP.subtract)
            vtt(rsy[:], s2v, sax[:], OP.mult)
            vtt(st1[:], c2v, say[:], OP.mult)
            vtt(rsy[:], rsy[:], st1[:], OP.add)
            inter16 = sm.tile([P, FB], f16, tag="inter16")
            vtt(inter16[:], dy16.h(0), rsx[:], OP.mult)
            vtt(st1[:], dx16.h(0), rsy[:], OP.mult)
            vtt(inter16[:], inter16[:], st1[:], OP.subtract)       # corr
            vtt(inter16[:], inter16[:], ps3[:, 0], OP.add)
            vtt(inter16[:], inter16[:], ps3[:, 1], OP.add)
            inter = sm.tile([P, FB], f32, tag="inter")
            S.activation(inter[:], inter16[:], AF.Relu, scale=0.5)  # inter area

            # ---- final loss (fp32) ----
            union = sm.tile([P, FB], f32, tag="union")
            fr1 = sm.tile([P, FB], f32, tag="fr1")
            iou = sm.tile([P, FB], f32, tag="iou")
            rr = sm.tile([P, FB], f32, tag="rr")
            lsa = sm.tile([P, 1], f32, tag="lsa")
            # union = 4*union0 - inter  (the *4 restores the /256 scale)
            V.scalar_tensor_tensor(union[:], union0[:], 4.0, inter[:],
                                   OP.mult, OP.subtract)
            V.reciprocal_approx_fast(out=fr1[:], in_=union[:])
            vtt(iou[:], inter[:], fr1[:], OP.mult)
            vts(iou[:], iou[:], 1e-6, None, OP.max)
            V.reciprocal_approx_fast(out=fr1[:], in_=area_c[:])
            vtt(fr1[:], union[:], fr1[:], OP.mult)
            vts(rr[:], fr1[:], -1.0, 1.0, OP.mult, OP.add)         # 1 - u/ac
            vtt(fr1[:], iou[:], iou[:], OP.mult)                   # iou^2
            vtt(fr1[:], fr1[:], iou[:], OP.mult)                   # iou^3
            vtt(iou[:], rr[:], rr[:], OP.mult)                     # rr^2
            vtt(iou[:], iou[:], rr[:], OP.mult)                    # rr^3
            vtt(fr1[:], fr1[:], iou[:], OP.subtract)               # giou
            V.tensor_reduce(lsa[:], fr1[:], AXL.X, OP.add)         # sum giou
            if debug:
                nc.sync.dma_start(out=dbg_d[0], in_=fr1[:])
                nc.sync.dma_start(out=dbg_d[1], in_=inter[:])
                nc.sync.dma_start(out=dbg_d[2], in_=union[:])
                nc.sync.dma_start(out=dbg_d[3], in_=area_c[:])
            nc.sync.dma_start(out=out_d[:], in_=lsa[:])

    nc.finalize()
    return nc


def _get_nc():
    if "nc" not in _CACHE:
        _CACHE["nc"] = _build()
    return _CACHE["nc"]


def _repack(pred, target):
    """Per-core input repack: planar rows so every SBUF slice is packed.
    ang/wh in fp16; xy quantized to int16 units of 1/32 px (diffs <= ~1500
    units stay exact in fp16). Rows beyond N_CORE are padded with identity
    boxes (w=h=16, a=0, same centers) whose giou is exactly 1."""
    in_maps = []
    for i in range(N_CORES):
        sl = slice(i * N_CORE, (i + 1) * N_CORE)
        p, t = pred[sl], target[sl]
        ang = np.zeros((2, NPAD), np.float16)
        ang[0, :N_CORE] = p[:, 4]
        ang[1, :N_CORE] = t[:, 4]
        # pads: concentric axis-aligned 16-box (pred) vs 8-box (target):
        # iou = 1/4, rr = 0 -> giou = 1/64 exactly (all fp16-exact values;
        # identical boxes would hit the coincident-boundary degeneracy)
        wh = np.empty((4, NPAD), np.float16)
        wh[0, N_CORE:] = 16.0
        wh[1, N_CORE:] = 8.0
        wh[2, N_CORE:] = 16.0
        wh[3, N_CORE:] = 8.0
        wh[0, :N_CORE] = p[:, 2]
        wh[1, :N_CORE] = t[:, 2]
        wh[2, :N_CORE] = p[:, 3]
        wh[3, :N_CORE] = t[:, 3]
        xy = np.full((4, NPAD), 16384, np.int16)
        for r, col in enumerate((p[:, 0], t[:, 0], p[:, 1], t[:, 1])):
            xy[r, :N_CORE] = np.clip(np.rint(col * XQ), 0, 32767).astype(np.int16)
        in_maps.append({"ang": ang, "wh": wh, "xy": xy})
    return in_maps


def kernel(pred, target):
    from concourse.bass_utils import run_bass_kernel_spmd

    pred = np.ascontiguousarray(np.asarray(pred, dtype=np.float32))
    target = np.ascontiguousarray(np.asarray(target, dtype=np.float32))
    nc = _get_nc()
    in_maps = _repack(pred, target)
    res = run_bass_kernel_spmd(nc, in_maps, core_ids=list(range(N_CORES)))
    gsum = np.float64(0.0)
    for i in range(N_CORES):
        gsum += np.asarray(res.results[i]["out"], dtype=np.float64).sum()
    # subtract the exact giou (=1/64) of the concentric pad boxes
    gsum -= float((NPAD - N_CORE) * N_CORES) * 0.015625
    # loss = mean(1 - giou) = 1 - sum(giou)/N
    return np.float32(1.0 - gsum / N_TOTAL)
